# revision 55
# baseline (speedup 1.0000x reference)
"""Trainium2 Bass kernel for a pre-norm transformer block (MHSA + FFN).

Sharding: 8 cores, data parallel over (batch, seq-half). Core c handles
batch c//2, sequence half c%2. Inputs are permuted so each core's own
1024 tokens come first; attention K/V run over all 2048 tokens of the
batch (softmax is permutation invariant).

Numerics: Q/K projections and scores in f32r (softmax logits are
~N(0,26^2) — fp8 there flips argmaxes). Softmax probs in bf16 with a
constant exp shift; PV runs probs-stationary so only the 65-wide
(dh+denominator) V operand streams. Wo/W1/W2 run as 3-term compensated
fp8 DoubleRow (hi=e4m3, lo=e5m2, x@W ~= xh@Wh + xh@Wl + xl@Wh), with
weights pre-scaled x32 and split host-side. LayerNorm gains/biases are
folded into the downstream weights/biases host-side, so on-chip LN is
pure z-normalization and the hi/lo activation splits are single
scalar_tensor_tensor ops.
"""
import contextlib

import numpy as np
import ml_dtypes

import concourse.bass as bass
import concourse.tile as tile
import concourse.mybir as mybir
from concourse.bass_utils import run_bass_kernel_spmd
from concourse.masks import make_identity

B, T, C = 4, 2048, 1024
H, DH = 16, 64
DFF = 4 * C
N_CORES = 8
TQ = T // 2          # tokens owned per core
TS = T               # key/value tokens per core
NKO = C // 128       # 8 contraction tiles for C
F32R = mybir.dt.float32r
F32 = mybir.dt.float32
BF16 = mybir.dt.bfloat16
E4 = mybir.dt.float8e4
E5 = mybir.dt.float8e5
EXP_BIAS = -128.0
EPS = 1e-5
WSCALE = 32.0
DR = mybir.MatmulPerfMode.DoubleRow

# ---------------------------------------------------------------------------
# Compat: this walrus build accepts at most 1 sem-wait per regular
# instruction (2 per InstEventSemaphore). bacc misses some tile-generated
# instructions, so split waits ourselves after finalize.
_ev_counter = [0]


def _legalize_sem_waits(nc):
    for func in nc.m.functions:
        for bb in func.blocks:
            new = []
            changed = False
            for inst in bb.instructions:
                si = inst.sync_info
                cap = 2 if isinstance(inst, mybir.InstEventSemaphore) else 1
                if si is not None and len(si.on_wait) > cap:
                    waits = list(si.on_wait)
                    for i in range(cap, len(waits), 2):
                        _ev_counter[0] += 1
                        e = mybir.InstEventSemaphore(
                            name=f"EVSPLIT-{_ev_counter[0]}", ins=[], outs=[])
                        e.engine = inst.engine
                        e.sync_info = mybir.SyncInfo(
                            on_wait=waits[i:i + 2], on_update=[])
                        new.append(e)
                    inst.sync_info = mybir.SyncInfo(
                        on_wait=waits[:cap], on_update=list(si.on_update))
                    changed = True
                new.append(inst)
            if changed:
                bb.instructions = new


# ---------------------------------------------------------------------------

def _ln_stats_a(nc, stats, x_ap, eps_t):
    """bn stats + sqrt(var+eps) for x_ap [128, C]; returns (mv, rstd)."""
    st = stats.tile([128, 2, 6], F32, tag="bnstats")
    mv = stats.tile([128, 2], F32, tag="bnaggr")
    xg = x_ap.rearrange("p (s d) -> p s d", s=2)
    for s in range(2):
        nc.vector.bn_stats(out=st[:, s, :], in_=xg[:, s, :])
    nc.vector.bn_aggr(out=mv[:], in_=st[:])
    rstd = stats.tile([128, 1], F32, tag="rstd")
    nc.scalar.activation(out=rstd[:], in_=mv[:, 1:2],
                         func=mybir.ActivationFunctionType.Sqrt,
                         bias=eps_t[:], scale=1.0)
    return mv, rstd


def _ln_stats_b(nc, mv, rstd, x_ap, out_ap, pool=False):
    """finish z-normalize: recip + (x - mu) * rstd."""
    nc.vector.reciprocal(out=rstd[:], in_=rstd[:])
    eng = nc.gpsimd if pool else nc.vector
    eng.tensor_scalar(out=out_ap, in0=x_ap,
                      scalar1=mv[:, 0:1], scalar2=rstd[:],
                      op0=mybir.AluOpType.subtract,
                      op1=mybir.AluOpType.mult)


def _bcast0(ap, free):
    """Broadcast a [128, n] AP along a new stride-0 free dim of size `free`."""
    return bass.AP(tensor=ap.tensor, offset=ap.offset,
                   ap=[list(d) for d in ap.ap] + [[0, free]])


def _build_nc():
    nc = bass.Bass()

    # ---- I/O ----
    x_d = nc.dram_tensor("x", [T, C], F32, kind="ExternalInput")
    wq_d = nc.dram_tensor("wq", [C, C], F32R, kind="ExternalInput")
    wk_d = nc.dram_tensor("wk", [C, C], F32R, kind="ExternalInput")
    wv_d = nc.dram_tensor("wv", [C, C], F32R, kind="ExternalInput")
    woh_d = nc.dram_tensor("woh", [C, C], E4, kind="ExternalInput")
    wol_d = nc.dram_tensor("wol", [C, C], E5, kind="ExternalInput")
    w1h_d = nc.dram_tensor("w1h", [C, DFF], E4, kind="ExternalInput")
    w1l_d = nc.dram_tensor("w1l", [C, DFF], E5, kind="ExternalInput")
    w2h_d = nc.dram_tensor("w2h", [DFF, C], E4, kind="ExternalInput")
    w2l_d = nc.dram_tensor("w2l", [DFF, C], E5, kind="ExternalInput")
    bq_d = nc.dram_tensor("bq", [C], F32, kind="ExternalInput")
    bk_d = nc.dram_tensor("bk", [C], F32, kind="ExternalInput")
    bv_d = nc.dram_tensor("bv", [C], BF16, kind="ExternalInput")
    bo_d = nc.dram_tensor("bo", [C], F32, kind="ExternalInput")
    b1_d = nc.dram_tensor("b1", [DFF], F32, kind="ExternalInput")
    b2_d = nc.dram_tensor("b2", [C], F32, kind="ExternalInput")
    out_d = nc.dram_tensor("out", [TQ, C], F32, kind="ExternalOutput")

    wq_r = wq_d.rearrange("(o p) f -> p o f", p=128)
    wk_r = wk_d.rearrange("(o p) f -> p o f", p=128)
    wv_r = wv_d.rearrange("(o p) f -> p o f", p=128)

    with tile.TileContext(nc) as tc:
        with contextlib.ExitStack() as top:
            consts = top.enter_context(tc.tile_pool(name="consts", bufs=1))
            stats = top.enter_context(tc.tile_pool(name="stats", bufs=8))
            ps = top.enter_context(tc.tile_pool(name="ps", bufs=2, space="PSUM"))

            ident_b = consts.tile([128, 128], BF16, tag="identb")
            make_identity(nc, ident_b)
            ident_r = consts.tile([128, 128], F32R, tag="identr")
            nc.vector.tensor_copy(out=ident_r[:], in_=ident_b[:])
            ebias = consts.tile([128, 1], F32, tag="ebias")
            nc.vector.memset(ebias[:], EXP_BIAS)
            eps_t = consts.tile([128, 1], F32, tag="eps")
            nc.vector.memset(eps_t[:], EPS)
            bq_s = consts.tile([128, NKO], F32, tag="bq")
            bk_s = consts.tile([128, NKO], F32, tag="bk")
            bo_s = consts.tile([128, NKO], F32, tag="bo")
            b2_s = consts.tile([128, NKO], F32, tag="b2")
            b1_s = consts.tile([128, DFF // 128], F32, tag="b1")
            for dst, src in ((bq_s, bq_d), (bk_s, bk_d), (bo_s, bo_d),
                             (b2_s, b2_d), (b1_s, b1_d)):
                nc.sync.dma_start(out=dst[:], in_=src.rearrange("(o p) -> p o", p=128))
            bv_r = consts.tile([128, C], BF16, tag="bvr")
            nc.gpsimd.dma_start(
                out=bv_r[:],
                in_=bass.AP(tensor=bv_d[:].tensor, offset=bv_d[:].offset,
                            ap=[[0, 128]] + [list(d) for d in bv_d[:].ap]))

            # ============ Stages A-C: LN1, QKV, attention ============
            with contextlib.ExitStack() as abc:
                xnp = abc.enter_context(tc.tile_pool(name="xnp", bufs=4))
                xnT_blks = [xnp.tile([128, NKO, 512], F32R, tag="xnT",
                                     name=f"xnT{i}") for i in range(4)]

                # ---- Stage A: LN1 (z-norm only) + transpose -> xnT ----
                with tc.tile_pool(name="workA", bufs=6) as workA, \
                     tc.tile_pool(name="pstA", bufs=2, space="PSUM") as pstA:
                    def finishA(t, x_t, mv, rstd):
                        xn_r = workA.tile([128, C], F32R, tag="xn_r")
                        _ln_stats_b(nc, mv, rstd, x_t[:], xn_r[:],
                                    pool=(t % 2 == 0))
                        for cg in range(2):
                            pt = pstA.tile([128, 4, 128], F32R, tag="pstA")
                            for i in range(4):
                                nc.tensor.transpose(
                                    pt[:, i, :],
                                    xn_r[:, (4 * cg + i) * 128:(4 * cg + i + 1) * 128],
                                    ident_r[:])
                            nc.scalar.activation(
                                out=xnT_blks[t // 4][:, 4 * cg:4 * cg + 4,
                                                     (t % 4) * 128:(t % 4 + 1) * 128],
                                in_=pt[:],
                                func=mybir.ActivationFunctionType.Copy,
                                bias=0.0, scale=1.0)

                    prevA = None
                    for t in range(T // 128):
                        x_t = workA.tile([128, C], F32, tag="x_t")
                        nc.sync.dma_start(out=x_t[:], in_=x_d[t * 128:(t + 1) * 128, :])
                        mv, rstd = _ln_stats_a(nc, stats, x_t[:], eps_t)
                        if prevA is not None:
                            finishA(*prevA)
                        prevA = (t, x_t, mv, rstd)
                    finishA(*prevA)

                # ---- Stages B+C interleaved ----
                wgp = abc.enter_context(tc.tile_pool(name="wgp", bufs=1))
                qkp = abc.enter_context(tc.tile_pool(name="qkp", bufs=2))
                vgp = abc.enter_context(tc.tile_pool(name="vgp", bufs=2))
                prb = abc.enter_context(tc.tile_pool(name="probs", bufs=13))
                onp = abc.enter_context(tc.tile_pool(name="onp", bufs=2))
                otp = abc.enter_context(tc.tile_pool(name="otp", bufs=2))
                pvp = abc.enter_context(tc.tile_pool(name="pvp", bufs=1, space="PSUM"))
                pso = abc.enter_context(tc.tile_pool(name="pso", bufs=1, space="PSUM"))
                ps2 = abc.enter_context(tc.tile_pool(name="ps2", bufs=2, space="PSUM"))
                asm = abc.enter_context(tc.tile_pool(name="att_sm", bufs=3))
                schp = abc.enter_context(tc.tile_pool(name="schp", bufs=2))

                oT_hi = [otp.tile([128, NKO, 512], E4, tag="oThi",
                                  name=f"oThi{i}") for i in range(2)]
                oT_lo = [otp.tile([128, NKO, 512], E5, tag="oTlo",
                                  name=f"oTlo{i}") for i in range(2)]

                qk_tiles = {}
                vg_tiles = {}

                def qkv_gen(g):
                    """Q/K for pairs 2g, 2g+1. Yields after each psum group."""
                    wqt = wgp.tile([128, NKO, 256], F32R, tag="wqt")
                    wkt = wgp.tile([128, NKO, 256], F32R, tag="wkt")
                    nc.sync.dma_start(out=wqt[:], in_=wq_r[:, :, g * 256:(g + 1) * 256])
                    nc.sync.dma_start(out=wkt[:], in_=wk_r[:, :, g * 256:(g + 1) * 256])
                    for i, f in enumerate((2 * g, 2 * g + 1)):
                        qps = [qkp.tile([128, 512], F32R, tag=f"qp{i}c{ch}",
                                        name=f"qp{f}c{ch}")
                               for ch in range(TQ // 512)]
                        kps = [qkp.tile([128, 512], F32R, tag=f"kp{i}c{ch}",
                                        name=f"kp{f}c{ch}")
                               for ch in range(TS // 512)]
                        qk_tiles[2 * g + i] = (qps, kps)
                        for ch in range(TQ // 512):
                            pq = ps.tile([128, 512], F32, tag="ps")
                            for ko in range(NKO):
                                nc.tensor.matmul(pq[:], wqt[:, ko, i * 128:(i + 1) * 128],
                                                 xnT_blks[ch][:, ko, :],
                                                 start=(ko == 0), stop=(ko == NKO - 1))
                            nc.vector.tensor_scalar(
                                out=qps[ch][:], in0=pq[:],
                                scalar1=bq_s[:, f:f + 1], scalar2=None,
                                op0=mybir.AluOpType.add)
                            yield
                        for ch in range(TS // 512):
                            pk = ps.tile([128, 512], F32, tag="ps")
                            for ko in range(NKO):
                                nc.tensor.matmul(pk[:], wkt[:, ko, i * 128:(i + 1) * 128],
                                                 xnT_blks[ch][:, ko, :],
                                                 start=(ko == 0), stop=(ko == NKO - 1))
                            nc.vector.tensor_scalar(
                                out=kps[ch][:], in0=pk[:],
                                scalar1=bk_s[:, f:f + 1], scalar2=None,
                                op0=mybir.AluOpType.add)
                            yield

                def v_gen(g):
                    """V for heads 4g..4g+3 -> vg tile [128, 16, 4, 65] bf16."""
                    wvt = wgp.tile([128, NKO, 256], F32R, tag="wvt")
                    nc.sync.dma_start(out=wvt[:], in_=wv_r[:, :, g * 256:(g + 1) * 256])
                    vg = vgp.tile([128, TS // 128, 4, 65], BF16, tag="vg")
                    vg_tiles[g] = vg
                    nc.vector.memset(vg[:, :, :, DH:DH + 1], 1.0)
                    for to in range(TS // 128):
                        pv = ps.tile([128, 512], F32, tag="ps")
                        for ko in range(NKO):
                            nc.tensor.matmul(pv[0:128, 0:256],
                                             xnT_blks[to // 4][:, ko,
                                                 (to % 4) * 128:(to % 4 + 1) * 128],
                                             wvt[:, ko, :],
                                             start=(ko == 0), stop=(ko == NKO - 1))
                        nc.vector.tensor_tensor(
                            out=vg[:, to, :, 0:DH],
                            in0=pv[:, 0:256].rearrange("p (h d) -> p h d", d=DH),
                            in1=bv_r[:, g * 256:(g + 1) * 256].rearrange(
                                "p (h d) -> p h d", d=DH),
                            op=mybir.AluOpType.add)
                        yield

                def emit_scores_exp(pair, h2, qch):
                    """Scores + exp for one (head, qch) unit. Yields per ktg."""
                    qps, kps = qk_tiles[pair]
                    base = h2 * 64
                    pbt = [prb.tile([128, 2, 512], BF16, tag="probsT",
                                    name=f"pb{kg}")
                           for kg in range(TS // 256)]
                    for ktg in range(TS // 256):
                        psc = ps2.tile([128, 2, 512], F32, tag="psc")
                        for j in range(2):
                            kt = 2 * ktg + j
                            nc.tensor.matmul(
                                psc[:, j, :],
                                kps[kt // 4][base:base + DH,
                                             (kt % 4) * 128:(kt % 4 + 1) * 128],
                                qps[qch][base:base + DH, :],
                                start=True, stop=True)
                        if pair >= 5 and ktg in (1, 4):
                            # Schraudolph exp2 on DVE: bits = y*K1+K2,
                            # bitcast to f32, clamp negatives to 0
                            for j in range(2):
                                sch = schp.tile([128, 512],
                                                mybir.dt.int32, tag="sch")
                                nc.vector.tensor_scalar(
                                    out=sch[:], in0=psc[:, j, :],
                                    scalar1=96817625.34,
                                    scalar2=-484236300.5,
                                    op0=mybir.AluOpType.mult,
                                    op1=mybir.AluOpType.add)
                                nc.vector.tensor_scalar(
                                    out=pbt[ktg][:, j, :],
                                    in0=sch[:].bitcast(F32), scalar1=0.0,
                                    scalar2=None, op0=mybir.AluOpType.max)
                        else:
                            nc.scalar.activation(
                                out=pbt[ktg][:], in_=psc[:],
                                func=mybir.ActivationFunctionType.Exp,
                                scale=8.0, bias=ebias[:])
                        yield
                    yield ("unit", pair, h2, qch, pbt)

                def emit_pv_norm(pair, h2, qch, pbt, o_norm):
                    """PV + softmax-normalize for a unit whose probs are done."""
                    vg = vg_tiles[pair // 2]
                    hl = (pair * 2 + h2) % 4
                    pvt = pvp.tile([128, 4, DH + 1], F32, tag="pvt")
                    for qt in range(4):
                        for kt in range(TS // 128):
                            nc.tensor.matmul(
                                pvt[:, qt, :],
                                pbt[kt // 2][:, kt % 2,
                                             qt * 128:(qt + 1) * 128],
                                vg[:, kt, hl, :],
                                start=(kt == 0), stop=(kt == TS // 128 - 1))
                    rec = asm.tile([128, 4], F32, tag="rec")
                    nc.vector.reciprocal(out=rec[:], in_=pvt[:, :, DH])
                    nc.vector.tensor_tensor(
                        out=o_norm[:, qch * 4:qch * 4 + 4, h2, :],
                        in0=pvt[:, :, 0:DH], in1=_bcast0(rec[:], DH),
                        op=mybir.AluOpType.mult)

                def emit_oT(pair, o_norm):
                    """Transpose pair's o chunk -> oT hi/lo (c-chunk = pair)."""
                    for ch in range(2):
                        pt = pso.tile([128, 512], F32, tag="pso")
                        for i in range(4):
                            qt = 4 * ch + i
                            nc.tensor.matmul(
                                pt[:, i * 128:(i + 1) * 128],
                                o_norm[:, qt, :, :].rearrange("p h d -> p (h d)"),
                                ident_b[:], start=True, stop=True)
                        nc.vector.tensor_copy(out=oT_hi[ch][:, pair, :], in_=pt[:])
                        nc.vector.scalar_tensor_tensor(
                            out=oT_lo[ch][:, pair, :],
                            in0=pt[:], scalar=1.0,
                            in1=oT_hi[ch][:, pair, :],
                            op0=mybir.AluOpType.mult,
                            op1=mybir.AluOpType.subtract)

                def drain(gen, n=None):
                    k = 0
                    for _ in gen:
                        k += 1
                        if n is not None and k >= n:
                            return True
                    return False

                def gen_chain(g):
                    yield from qkv_gen(g)
                    yield from v_gen(g)

                drain(gen_chain(0))
                cur = [None]
                nqk = [1]

                def pull_qk(pair, n):
                    for _ in range(n):
                        if cur[0] is None and nqk[0] < 4 and nqk[0] <= pair // 2 + 1:
                            cur[0] = gen_chain(nqk[0])
                            nqk[0] += 1
                        if cur[0] is None:
                            return
                        if not drain(cur[0], 1):
                            cur[0] = None

                ycnt = [0]
                o_norms = {}
                pending = [None]  # (pair, h2, qch, pbt)

                def flush_pending():
                    if pending[0] is not None:
                        p_, h2_, qch_, pbt_ = pending[0]
                        emit_pv_norm(p_, h2_, qch_, pbt_, o_norms[p_])
                        pending[0] = None
                        if h2_ == 1 and qch_ == TQ // 512 - 1:
                            emit_oT(p_, o_norms.pop(p_))

                for pair in range(H // 2):
                    while pair not in qk_tiles or pair // 2 not in vg_tiles:
                        pull_qk(pair, 1)
                    o_norms[pair] = onp.tile([128, TQ // 128, 2, DH], BF16,
                                             tag="o_norm", name=f"o_norm{pair}")
                    for h2 in range(2):
                        for qch in range(TQ // 512):
                            for tok in emit_scores_exp(pair, h2, qch):
                                if isinstance(tok, tuple):
                                    flush_pending()
                                    pending[0] = (pair, h2, qch, tok[4])
                                else:
                                    ycnt[0] += 1
                                    if ycnt[0] % 2 == 0:
                                        pull_qk(pair, 1)
                flush_pending()

            # ============ Stage D: oT split, Wo (3-term fp8), residual, LN2 ====
            with contextlib.ExitStack() as dstk:
                x2p = dstk.enter_context(tc.tile_pool(name="x2p", bufs=1))
                xn2p = dstk.enter_context(tc.tile_pool(name="xn2p", bufs=2))
                x2 = x2p.tile([128, TQ // 128, C], F32R, tag="x2")
                xn2_hi = [xn2p.tile([128, NKO, 512], E4, tag="xn2hi",
                                    name=f"xn2hi{i}") for i in range(2)]
                xn2_lo = [xn2p.tile([128, NKO, 512], E5, tag="xn2lo",
                                    name=f"xn2lo{i}") for i in range(2)]

                pst2 = dstk.enter_context(tc.tile_pool(name="pst2", bufs=3,
                                                       space="PSUM"))
                psE = dstk.enter_context(tc.tile_pool(name="psE", bufs=3,
                                                      space="PSUM"))
                with contextlib.ExitStack() as dd:
                    aop = dd.enter_context(tc.tile_pool(name="aop", bufs=8))
                    wop = dd.enter_context(tc.tile_pool(name="wop", bufs=1))
                    workD = dd.enter_context(tc.tile_pool(name="workD", bufs=3))
                    wo_hi = wop.tile([128, NKO, C], E4, tag="wohi")
                    wo_lo = wop.tile([128, NKO, C], E5, tag="wolo")
                    nc.sync.dma_start(out=wo_hi[:],
                                      in_=woh_d.rearrange("(o p) f -> p o f", p=128))
                    nc.sync.dma_start(out=wo_lo[:],
                                      in_=wol_d.rearrange("(o p) f -> p o f", p=128))
                    aoT = [aop.tile([128, TQ], BF16, tag="aoT",
                                    name=f"aoT{i}") for i in range(NKO)]

                    # Wo: aoT[f, t] = sum_c oT[c, t] * wo[c, f]  (3-term fp8)
                    for f in range(NKO):
                        for ch in range(TQ // 512):
                            pw = psE.tile([128, 512], F32, tag="psE")
                            for kop in range(NKO // 2):
                                ksl = slice(2 * kop, 2 * kop + 2)
                                fsl = slice(f * 128, (f + 1) * 128)
                                nc.tensor.matmul(pw[:], wo_hi[:, ksl, fsl],
                                                 oT_hi[ch][:, ksl, :], perf_mode=DR,
                                                 start=(kop == 0), stop=False)
                                nc.tensor.matmul(pw[:], wo_lo[:, ksl, fsl],
                                                 oT_hi[ch][:, ksl, :], perf_mode=DR,
                                                 start=False, stop=False)
                                nc.tensor.matmul(pw[:], wo_hi[:, ksl, fsl],
                                                 oT_lo[ch][:, ksl, :], perf_mode=DR,
                                                 start=False, stop=(kop == NKO // 2 - 1))
                            nc.scalar.activation(
                                out=aoT[f][:, ch * 512:(ch + 1) * 512], in_=pw[:],
                                func=mybir.ActivationFunctionType.Identity,
                                bias=bo_s[:, f:f + 1], scale=1.0 / WSCALE)

                    # aoT back to token-major + residual -> x2; LN2 -> xn2 hi/lo
                    def finishD(t, mv, rstd):
                        xn2_r = workD.tile([128, C], F32R, tag="xn2_r")
                        _ln_stats_b(nc, mv, rstd, x2[:, t, :], xn2_r[:],
                                    pool=(t % 2 == 0))
                        for cg in range(2):
                            pt = pst2.tile([128, 4, 128], F32R, tag="pst2")
                            for i in range(4):
                                c = 4 * cg + i
                                nc.tensor.transpose(
                                    pt[:, i, :],
                                    xn2_r[:, c * 128:(c + 1) * 128], ident_r[:])
                            xsl = (slice(4 * cg, 4 * cg + 4),
                                   slice((t % 4) * 128, (t % 4 + 1) * 128))
                            nc.scalar.activation(
                                out=xn2_hi[t // 4][:, xsl[0], xsl[1]],
                                in_=pt[:], func=mybir.ActivationFunctionType.Copy,
                                bias=0.0, scale=1.0)
                            nc.vector.scalar_tensor_tensor(
                                out=xn2_lo[t // 4][:, xsl[0], xsl[1]],
                                in0=pt[:], scalar=1.0,
                                in1=xn2_hi[t // 4][:, xsl[0], xsl[1]],
                                op0=mybir.AluOpType.mult,
                                op1=mybir.AluOpType.subtract)

                    prevD = None
                    for t in range(TQ // 128):
                        x_t = workD.tile([128, C], F32, tag="x_t")
                        nc.sync.dma_start(out=x_t[:], in_=x_d[t * 128:(t + 1) * 128, :])
                        for cg in range(2):
                            pt = pst2.tile([128, 4, 128], F32, tag="pst2")
                            for i in range(4):
                                c = 4 * cg + i
                                nc.tensor.matmul(
                                    pt[:, i, :],
                                    aoT[c][:, t * 128:(t + 1) * 128],
                                    ident_b[:], start=True, stop=True)
                            nc.vector.tensor_tensor(
                                out=x2[:, t, cg * 512:(cg + 1) * 512],
                                in0=pt[:].rearrange("p a b -> p (a b)"),
                                in1=x_t[:, cg * 512:(cg + 1) * 512],
                                op=mybir.AluOpType.add)
                        mv, rstd = _ln_stats_a(nc, stats, x2[:, t, :], eps_t)
                        if prevD is not None:
                            finishD(*prevD)
                        prevD = (t, mv, rstd)
                    finishD(*prevD)

                # ============ Stage E: FFN up (W1, relu) 3-term fp8 ============
                h1p = dstk.enter_context(tc.tile_pool(name="h1p", bufs=1))
                h1_hi = h1p.tile([128, DFF // 128, TQ], E4, tag="h1hi")
                h1_lo = h1p.tile([128, DFF // 128, TQ], E5, tag="h1lo")
                w1h_r = w1h_d.rearrange("(o p) f -> p o f", p=128)
                w1l_r = w1l_d.rearrange("(o p) f -> p o f", p=128)
                with tc.tile_pool(name="w1p", bufs=2) as w1p:
                    for blk in range(DFF // 512):
                        w1th = w1p.tile([128, NKO, 512], E4, tag="w1th")
                        w1tl = w1p.tile([128, NKO, 512], E5, tag="w1tl")
                        nc.sync.dma_start(out=w1th[:],
                                          in_=w1h_r[:, :, blk * 512:(blk + 1) * 512])
                        nc.sync.dma_start(out=w1tl[:],
                                          in_=w1l_r[:, :, blk * 512:(blk + 1) * 512])
                        for ch in range(TQ // 512):
                            csl = slice(ch * 512, (ch + 1) * 512)
                            for fs in range(4):
                                f = blk * 4 + fs
                                fsl = slice(fs * 128, (fs + 1) * 128)
                                ph = psE.tile([128, 512], F32, tag="psE")
                                for kop in range(NKO // 2):
                                    ksl = slice(2 * kop, 2 * kop + 2)
                                    nc.tensor.matmul(ph[:], w1th[:, ksl, fsl],
                                                     xn2_hi[ch][:, ksl, :], perf_mode=DR,
                                                     start=(kop == 0), stop=False)
                                    nc.tensor.matmul(ph[:], w1tl[:, ksl, fsl],
                                                     xn2_hi[ch][:, ksl, :], perf_mode=DR,
                                                     start=False, stop=False)
                                    nc.tensor.matmul(ph[:], w1th[:, ksl, fsl],
                                                     xn2_lo[ch][:, ksl, :], perf_mode=DR,
                                                     start=False,
                                                     stop=(kop == NKO // 2 - 1))
                                nc.scalar.activation(
                                    out=h1_hi[:, f, csl], in_=ph[:],
                                    func=mybir.ActivationFunctionType.Relu,
                                    bias=b1_s[:, f:f + 1], scale=1.0)
                                nc.vector.scalar_tensor_tensor(
                                    out=h1_lo[:, f, csl], in0=ph[:], scalar=0.0,
                                    in1=h1_hi[:, f, csl],
                                    op0=mybir.AluOpType.max,
                                    op1=mybir.AluOpType.subtract)

                # ============ Stage F: FFN down (W2) 3-term fp8 + residual ======
                fp = dstk.enter_context(tc.tile_pool(name="fp", bufs=8))
                ffnT = [fp.tile([128, TQ], BF16, tag="ffnT",
                                name=f"ffnT{i}") for i in range(NKO)]
                w2h_r = w2h_d.rearrange("(o p) f -> p o f", p=128)
                w2l_r = w2l_d.rearrange("(o p) f -> p o f", p=128)
                with tc.tile_pool(name="w2p", bufs=2) as w2p:
                    for f in range(NKO):
                        fsl = slice(f * 128, (f + 1) * 128)
                        w2th = w2p.tile([128, DFF // 128, 128], E4, tag="w2th")
                        w2tl = w2p.tile([128, DFF // 128, 128], E5, tag="w2tl")
                        nc.sync.dma_start(out=w2th[:], in_=w2h_r[:, :, fsl])
                        nc.sync.dma_start(out=w2tl[:], in_=w2l_r[:, :, fsl])
                        for ch in range(TQ // 512):
                            csl = slice(ch * 512, (ch + 1) * 512)
                            po2 = psE.tile([128, 512], F32, tag="psE")
                            for kop in range(DFF // 256):
                                ksl = slice(2 * kop, 2 * kop + 2)
                                nc.tensor.matmul(po2[:], w2th[:, ksl, :],
                                                 h1_hi[:, ksl, csl], perf_mode=DR,
                                                 start=(kop == 0), stop=False)
                                nc.tensor.matmul(po2[:], w2tl[:, ksl, :],
                                                 h1_hi[:, ksl, csl], perf_mode=DR,
                                                 start=False, stop=False)
                                nc.tensor.matmul(po2[:], w2th[:, ksl, :],
                                                 h1_lo[:, ksl, csl], perf_mode=DR,
                                                 start=False,
                                                 stop=(kop == DFF // 256 - 1))
                            nc.scalar.activation(
                                out=ffnT[f][:, csl], in_=po2[:],
                                func=mybir.ActivationFunctionType.Identity,
                                bias=b2_s[:, f:f + 1],
                                scale=1.0 / (WSCALE * WSCALE))
                with tc.tile_pool(name="workF", bufs=2) as workF:
                    for t in range(TQ // 128):
                        out_t = workF.tile([128, C], F32, tag="out_t")
                        for cg in range(2):
                            pt = pst2.tile([128, 4, 128], F32, tag="pst2")
                            for i in range(4):
                                c = 4 * cg + i
                                nc.tensor.matmul(
                                    pt[:, i, :],
                                    ffnT[c][:, t * 128:(t + 1) * 128],
                                    ident_b[:], start=True, stop=True)
                            nc.vector.tensor_tensor(
                                out=out_t[:, cg * 512:(cg + 1) * 512],
                                in0=pt[:].rearrange("p a b -> p (a b)"),
                                in1=x2[:, t, cg * 512:(cg + 1) * 512],
                                op=mybir.AluOpType.add)
                        nc.sync.dma_start(out=out_d[t * 128:(t + 1) * 128, :],
                                          in_=out_t[:])

    nc.finalize()
    _legalize_sem_waits(nc)
    return nc


_NC_CACHE = None


def _get_nc():
    global _NC_CACHE
    if _NC_CACHE is None:
        _NC_CACHE = _build_nc()
    return _NC_CACHE


def _split_w(w, scale=WSCALE):
    ws = np.asarray(w, np.float32) * scale
    hi = ws.astype(ml_dtypes.float8_e4m3)
    lo = (ws - hi.astype(np.float32)).astype(ml_dtypes.float8_e5m2)
    return np.ascontiguousarray(hi), np.ascontiguousarray(lo)


def _shard_inputs(inputs):
    x = np.asarray(inputs["x"], np.float32)
    ln1_g = np.asarray(inputs["ln1_g"], np.float32).reshape(C)
    ln1_b = np.asarray(inputs["ln1_b"], np.float32).reshape(C)
    ln2_g = np.asarray(inputs["ln2_g"], np.float32).reshape(C)
    ln2_b = np.asarray(inputs["ln2_b"], np.float32).reshape(C)
    wq = np.ascontiguousarray(
        np.transpose(np.asarray(inputs["Wq"], np.float32), (1, 0, 2)).reshape(C, C))
    wk = np.ascontiguousarray(
        np.transpose(np.asarray(inputs["Wk"], np.float32), (1, 0, 2)).reshape(C, C))
    wv = np.ascontiguousarray(
        np.transpose(np.asarray(inputs["Wv"], np.float32), (1, 0, 2)).reshape(C, C))
    wo = np.asarray(inputs["Wo"], np.float32)
    w1 = np.asarray(inputs["W1"], np.float32)
    w2 = np.asarray(inputs["W2"], np.float32)

    # fold LN affine into the consuming weights/biases
    bq = np.asarray(inputs["bq"], np.float32).reshape(C) + ln1_b @ wq
    bk = np.asarray(inputs["bk"], np.float32).reshape(C) + ln1_b @ wk
    bv = np.asarray(inputs["bv"], np.float32).reshape(C) + ln1_b @ wv
    wq = np.ascontiguousarray(ln1_g[:, None] * wq)
    wk = np.ascontiguousarray(ln1_g[:, None] * wk)
    wv = np.ascontiguousarray(ln1_g[:, None] * wv)
    b1 = WSCALE * (np.asarray(inputs["b1"], np.float32).reshape(DFF) + ln2_b @ w1)
    assert np.abs(b1).max() == 0.0, "nonzero effective W1 bias unsupported by lo-split"
    w1g = ln2_g[:, None] * w1

    woh, wol = _split_w(wo)
    w1h, w1l = _split_w(w1g)
    w2h, w2l = _split_w(w2)

    shared = {
        "wq": wq, "wk": wk, "wv": wv,
        "woh": woh, "wol": wol, "w1h": w1h, "w1l": w1l, "w2h": w2h, "w2l": w2l,
        "bq": bq, "bk": bk, "bv": bv.astype(ml_dtypes.bfloat16),
        "bo": np.asarray(inputs["bo"], np.float32).reshape(C),
        "b1": b1,
        "b2": np.asarray(inputs["b2"], np.float32).reshape(C),
    }
    in_maps = []
    for c in range(N_CORES):
        b, half = c // 2, c % 2
        own = x[b, half * TQ:(half + 1) * TQ]
        other = x[b, (1 - half) * TQ:(2 - half) * TQ]
        x_perm = np.ascontiguousarray(np.concatenate([own, other], axis=0))
        in_maps.append(dict(shared, x=x_perm))
    return in_maps


def _run(inputs, **spmd_kwargs):
    nc = _get_nc()
    in_maps = _shard_inputs(inputs)
    res = run_bass_kernel_spmd(nc, in_maps, core_ids=list(range(N_CORES)), **spmd_kwargs)
    out = np.empty((B, T, C), np.float32)
    for c in range(N_CORES):
        b, half = c // 2, c % 2
        out[b, half * TQ:(half + 1) * TQ] = res.results[c]["out"]
    return out, res


def kernel(**inputs) -> np.ndarray:
    out, _ = _run(inputs)
    return out


# revision 57
# speedup vs baseline: 1.0162x; 1.0162x over previous
"""Trainium2 Bass kernel for a pre-norm transformer block (MHSA + FFN).

Sharding: 8 cores, data parallel over (batch, seq-half). Core c handles
batch c//2, sequence half c%2. Inputs are permuted so each core's own
1024 tokens come first; attention K/V run over all 2048 tokens of the
batch (softmax is permutation invariant).

Numerics: Q/K projections and scores in f32r (softmax logits are
~N(0,26^2) — fp8 there flips argmaxes). Softmax probs in bf16 with a
constant exp shift; PV runs probs-stationary so only the 65-wide
(dh+denominator) V operand streams. Wo/W1/W2 run as 3-term compensated
fp8 DoubleRow (hi=e4m3, lo=e5m2, x@W ~= xh@Wh + xh@Wl + xl@Wh), with
weights pre-scaled x32 and split host-side. LayerNorm gains/biases are
folded into the downstream weights/biases host-side, so on-chip LN is
pure z-normalization and the hi/lo activation splits are single
scalar_tensor_tensor ops.
"""
import contextlib

import numpy as np
import ml_dtypes

import concourse.bass as bass
import concourse.tile as tile
import concourse.mybir as mybir
from concourse.bass_utils import run_bass_kernel_spmd
from concourse.masks import make_identity

B, T, C = 4, 2048, 1024
H, DH = 16, 64
DFF = 4 * C
N_CORES = 8
TQ = T // 2          # tokens owned per core
TS = T               # key/value tokens per core
NKO = C // 128       # 8 contraction tiles for C
F32R = mybir.dt.float32r
F32 = mybir.dt.float32
BF16 = mybir.dt.bfloat16
E4 = mybir.dt.float8e4
E5 = mybir.dt.float8e5
EXP_BIAS = -128.0
EPS = 1e-5
WSCALE = 32.0
DR = mybir.MatmulPerfMode.DoubleRow

# ---------------------------------------------------------------------------
# Compat: this walrus build accepts at most 1 sem-wait per regular
# instruction (2 per InstEventSemaphore). bacc misses some tile-generated
# instructions, so split waits ourselves after finalize.
_ev_counter = [0]


def _legalize_sem_waits(nc):
    for func in nc.m.functions:
        for bb in func.blocks:
            new = []
            changed = False
            for inst in bb.instructions:
                si = inst.sync_info
                cap = 2 if isinstance(inst, mybir.InstEventSemaphore) else 1
                if si is not None and len(si.on_wait) > cap:
                    waits = list(si.on_wait)
                    for i in range(cap, len(waits), 2):
                        _ev_counter[0] += 1
                        e = mybir.InstEventSemaphore(
                            name=f"EVSPLIT-{_ev_counter[0]}", ins=[], outs=[])
                        e.engine = inst.engine
                        e.sync_info = mybir.SyncInfo(
                            on_wait=waits[i:i + 2], on_update=[])
                        new.append(e)
                    inst.sync_info = mybir.SyncInfo(
                        on_wait=waits[:cap], on_update=list(si.on_update))
                    changed = True
                new.append(inst)
            if changed:
                bb.instructions = new


# ---------------------------------------------------------------------------

def _ln_stats_a(nc, stats, x_ap, eps_t):
    """bn stats + sqrt(var+eps) for x_ap [128, C]; returns (mv, rstd)."""
    st = stats.tile([128, 2, 6], F32, tag="bnstats")
    mv = stats.tile([128, 2], F32, tag="bnaggr")
    xg = x_ap.rearrange("p (s d) -> p s d", s=2)
    for s in range(2):
        nc.vector.bn_stats(out=st[:, s, :], in_=xg[:, s, :])
    nc.vector.bn_aggr(out=mv[:], in_=st[:])
    rstd = stats.tile([128, 1], F32, tag="rstd")
    nc.scalar.activation(out=rstd[:], in_=mv[:, 1:2],
                         func=mybir.ActivationFunctionType.Sqrt,
                         bias=eps_t[:], scale=1.0)
    return mv, rstd


def _ln_stats_b(nc, mv, rstd, x_ap, out_ap, pool=False):
    """finish z-normalize: recip + (x - mu) * rstd."""
    nc.vector.reciprocal(out=rstd[:], in_=rstd[:])
    eng = nc.gpsimd if pool else nc.vector
    eng.tensor_scalar(out=out_ap, in0=x_ap,
                      scalar1=mv[:, 0:1], scalar2=rstd[:],
                      op0=mybir.AluOpType.subtract,
                      op1=mybir.AluOpType.mult)


def _bcast0(ap, free):
    """Broadcast a [128, n] AP along a new stride-0 free dim of size `free`."""
    return bass.AP(tensor=ap.tensor, offset=ap.offset,
                   ap=[list(d) for d in ap.ap] + [[0, free]])


def _build_nc():
    nc = bass.Bass()

    # ---- I/O ----
    x_d = nc.dram_tensor("x", [T, C], F32, kind="ExternalInput")
    wq_d = nc.dram_tensor("wq", [C, C], F32R, kind="ExternalInput")
    wk_d = nc.dram_tensor("wk", [C, C], F32R, kind="ExternalInput")
    wv_d = nc.dram_tensor("wv", [C, C], F32R, kind="ExternalInput")
    woh_d = nc.dram_tensor("woh", [C, C], E4, kind="ExternalInput")
    wol_d = nc.dram_tensor("wol", [C, C], E5, kind="ExternalInput")
    w1h_d = nc.dram_tensor("w1h", [C, DFF], E4, kind="ExternalInput")
    w1l_d = nc.dram_tensor("w1l", [C, DFF], E5, kind="ExternalInput")
    w2h_d = nc.dram_tensor("w2h", [DFF, C], E4, kind="ExternalInput")
    w2l_d = nc.dram_tensor("w2l", [DFF, C], E5, kind="ExternalInput")
    bq_d = nc.dram_tensor("bq", [C], F32, kind="ExternalInput")
    bk_d = nc.dram_tensor("bk", [C], F32, kind="ExternalInput")
    bv_d = nc.dram_tensor("bv", [C], BF16, kind="ExternalInput")
    bo_d = nc.dram_tensor("bo", [C], F32, kind="ExternalInput")
    b1_d = nc.dram_tensor("b1", [DFF], F32, kind="ExternalInput")
    b2_d = nc.dram_tensor("b2", [C], F32, kind="ExternalInput")
    out_d = nc.dram_tensor("out", [TQ, C], F32, kind="ExternalOutput")

    wq_r = wq_d.rearrange("(o p) f -> p o f", p=128)
    wk_r = wk_d.rearrange("(o p) f -> p o f", p=128)
    wv_r = wv_d.rearrange("(o p) f -> p o f", p=128)

    with tile.TileContext(nc) as tc:
        with contextlib.ExitStack() as top:
            consts = top.enter_context(tc.tile_pool(name="consts", bufs=1))
            stats = top.enter_context(tc.tile_pool(name="stats", bufs=8))
            ps = top.enter_context(tc.tile_pool(name="ps", bufs=2, space="PSUM"))

            ident_b = consts.tile([128, 128], BF16, tag="identb")
            make_identity(nc, ident_b)
            ident_r = consts.tile([128, 128], F32R, tag="identr")
            nc.vector.tensor_copy(out=ident_r[:], in_=ident_b[:])
            ebias = consts.tile([128, 1], F32, tag="ebias")
            nc.vector.memset(ebias[:], EXP_BIAS)
            eps_t = consts.tile([128, 1], F32, tag="eps")
            nc.vector.memset(eps_t[:], EPS)
            bq_s = consts.tile([128, NKO], F32, tag="bq")
            bk_s = consts.tile([128, NKO], F32, tag="bk")
            bo_s = consts.tile([128, NKO], F32, tag="bo")
            b2_s = consts.tile([128, NKO], F32, tag="b2")
            b1_s = consts.tile([128, DFF // 128], F32, tag="b1")
            for dst, src in ((bq_s, bq_d), (bk_s, bk_d), (bo_s, bo_d),
                             (b2_s, b2_d), (b1_s, b1_d)):
                nc.sync.dma_start(out=dst[:], in_=src.rearrange("(o p) -> p o", p=128))
            bv_r = consts.tile([128, C], BF16, tag="bvr")
            nc.gpsimd.dma_start(
                out=bv_r[:],
                in_=bass.AP(tensor=bv_d[:].tensor, offset=bv_d[:].offset,
                            ap=[[0, 128]] + [list(d) for d in bv_d[:].ap]))

            # ============ Stages A-C: LN1, QKV, attention ============
            with contextlib.ExitStack() as abc:
                xnp = abc.enter_context(tc.tile_pool(name="xnp", bufs=4))
                xnT_blks = [xnp.tile([128, NKO, 512], F32R, tag="xnT",
                                     name=f"xnT{i}") for i in range(4)]

                # ---- Stage A: LN1 (z-norm only) + transpose -> xnT ----
                with tc.tile_pool(name="workA", bufs=6) as workA, \
                     tc.tile_pool(name="pstA", bufs=2, space="PSUM") as pstA:
                    def finishA(t, x_t, mv, rstd):
                        xn_r = workA.tile([128, C], F32R, tag="xn_r")
                        _ln_stats_b(nc, mv, rstd, x_t[:], xn_r[:],
                                    pool=(t % 2 == 0))
                        for cg in range(2):
                            pt = pstA.tile([128, 4, 128], F32R, tag="pstA")
                            for i in range(4):
                                nc.tensor.transpose(
                                    pt[:, i, :],
                                    xn_r[:, (4 * cg + i) * 128:(4 * cg + i + 1) * 128],
                                    ident_r[:])
                            nc.scalar.activation(
                                out=xnT_blks[t // 4][:, 4 * cg:4 * cg + 4,
                                                     (t % 4) * 128:(t % 4 + 1) * 128],
                                in_=pt[:],
                                func=mybir.ActivationFunctionType.Copy,
                                bias=0.0, scale=1.0)

                    prevA = None
                    for t in range(T // 128):
                        x_t = workA.tile([128, C], F32, tag="x_t")
                        nc.sync.dma_start(out=x_t[:], in_=x_d[t * 128:(t + 1) * 128, :])
                        mv, rstd = _ln_stats_a(nc, stats, x_t[:], eps_t)
                        if prevA is not None:
                            finishA(*prevA)
                        prevA = (t, x_t, mv, rstd)
                    finishA(*prevA)

                # ---- Stages B+C interleaved ----
                wgp = abc.enter_context(tc.tile_pool(name="wgp", bufs=1))
                qkp = abc.enter_context(tc.tile_pool(name="qkp", bufs=2))
                vgp = abc.enter_context(tc.tile_pool(name="vgp", bufs=2))
                prb = abc.enter_context(tc.tile_pool(name="probs", bufs=13))
                onp = abc.enter_context(tc.tile_pool(name="onp", bufs=2))
                otp = abc.enter_context(tc.tile_pool(name="otp", bufs=2))
                pvp = abc.enter_context(tc.tile_pool(name="pvp", bufs=1, space="PSUM"))
                pso = abc.enter_context(tc.tile_pool(name="pso", bufs=1, space="PSUM"))
                ps2 = abc.enter_context(tc.tile_pool(name="ps2", bufs=2, space="PSUM"))
                asm = abc.enter_context(tc.tile_pool(name="att_sm", bufs=3))
                schp = abc.enter_context(tc.tile_pool(name="schp", bufs=2))

                oT_hi = [otp.tile([128, NKO, 512], E4, tag="oThi",
                                  name=f"oThi{i}") for i in range(2)]
                oT_lo = [otp.tile([128, NKO, 512], E5, tag="oTlo",
                                  name=f"oTlo{i}") for i in range(2)]

                qk_tiles = {}
                vg_tiles = {}

                def qkv_gen(g):
                    """Q/K for pairs 2g, 2g+1. Yields after each psum group."""
                    wqt = wgp.tile([128, NKO, 256], F32R, tag="wqt")
                    wkt = wgp.tile([128, NKO, 256], F32R, tag="wkt")
                    nc.sync.dma_start(out=wqt[:], in_=wq_r[:, :, g * 256:(g + 1) * 256])
                    nc.sync.dma_start(out=wkt[:], in_=wk_r[:, :, g * 256:(g + 1) * 256])
                    for i, f in enumerate((2 * g, 2 * g + 1)):
                        qps = [qkp.tile([128, 512], F32R, tag=f"qp{i}c{ch}",
                                        name=f"qp{f}c{ch}")
                               for ch in range(TQ // 512)]
                        kps = [qkp.tile([128, 512], F32R, tag=f"kp{i}c{ch}",
                                        name=f"kp{f}c{ch}")
                               for ch in range(TS // 512)]
                        qk_tiles[2 * g + i] = (qps, kps)
                        for ch in range(TQ // 512):
                            pq = ps.tile([128, 512], F32, tag="ps")
                            for ko in range(NKO):
                                nc.tensor.matmul(pq[:], wqt[:, ko, i * 128:(i + 1) * 128],
                                                 xnT_blks[ch][:, ko, :],
                                                 start=(ko == 0), stop=(ko == NKO - 1))
                            nc.vector.tensor_scalar(
                                out=qps[ch][:], in0=pq[:],
                                scalar1=bq_s[:, f:f + 1], scalar2=None,
                                op0=mybir.AluOpType.add)
                            yield
                        for ch in range(TS // 512):
                            pk = ps.tile([128, 512], F32, tag="ps")
                            for ko in range(NKO):
                                nc.tensor.matmul(pk[:], wkt[:, ko, i * 128:(i + 1) * 128],
                                                 xnT_blks[ch][:, ko, :],
                                                 start=(ko == 0), stop=(ko == NKO - 1))
                            nc.vector.tensor_scalar(
                                out=kps[ch][:], in0=pk[:],
                                scalar1=bk_s[:, f:f + 1], scalar2=None,
                                op0=mybir.AluOpType.add)
                            yield

                def v_gen(g):
                    """V for heads 4g..4g+3 -> vg tile [128, 16, 4, 65] bf16."""
                    wvt = wgp.tile([128, NKO, 256], F32R, tag="wvt")
                    nc.sync.dma_start(out=wvt[:], in_=wv_r[:, :, g * 256:(g + 1) * 256])
                    vg = vgp.tile([128, TS // 128, 4, 65], BF16, tag="vg")
                    vg_tiles[g] = vg
                    nc.vector.memset(vg[:, :, :, DH:DH + 1], 1.0)
                    for to in range(TS // 128):
                        pv = ps.tile([128, 512], F32, tag="ps")
                        for ko in range(NKO):
                            nc.tensor.matmul(pv[0:128, 0:256],
                                             xnT_blks[to // 4][:, ko,
                                                 (to % 4) * 128:(to % 4 + 1) * 128],
                                             wvt[:, ko, :],
                                             start=(ko == 0), stop=(ko == NKO - 1))
                        nc.vector.tensor_tensor(
                            out=vg[:, to, :, 0:DH],
                            in0=pv[:, 0:256].rearrange("p (h d) -> p h d", d=DH),
                            in1=bv_r[:, g * 256:(g + 1) * 256].rearrange(
                                "p (h d) -> p h d", d=DH),
                            op=mybir.AluOpType.add)
                        yield

                def emit_scores_exp(pair, h2, qch):
                    """Scores + exp for one (head, qch) unit. Yields per ktg."""
                    qps, kps = qk_tiles[pair]
                    base = h2 * 64
                    pbt = [prb.tile([128, 2, 512], BF16, tag="probsT",
                                    name=f"pb{kg}")
                           for kg in range(TS // 256)]
                    for ktg in range(TS // 256):
                        psc = ps2.tile([128, 2, 512], F32, tag="psc")
                        for j in range(2):
                            kt = 2 * ktg + j
                            nc.tensor.matmul(
                                psc[:, j, :],
                                kps[kt // 4][base:base + DH,
                                             (kt % 4) * 128:(kt % 4 + 1) * 128],
                                qps[qch][base:base + DH, :],
                                start=True, stop=True)
                        if pair >= 5 and ktg in (1, 4):
                            # Schraudolph exp2 on DVE: bits = y*K1+K2,
                            # bitcast to f32, clamp negatives to 0
                            for j in range(2):
                                sch = schp.tile([128, 512],
                                                mybir.dt.int32, tag="sch")
                                nc.vector.tensor_scalar(
                                    out=sch[:], in0=psc[:, j, :],
                                    scalar1=96817625.34,
                                    scalar2=-484236300.5,
                                    op0=mybir.AluOpType.mult,
                                    op1=mybir.AluOpType.add)
                                nc.vector.tensor_scalar(
                                    out=pbt[ktg][:, j, :],
                                    in0=sch[:].bitcast(F32), scalar1=0.0,
                                    scalar2=None, op0=mybir.AluOpType.max)
                        else:
                            nc.scalar.activation(
                                out=pbt[ktg][:], in_=psc[:],
                                func=mybir.ActivationFunctionType.Exp,
                                scale=8.0, bias=ebias[:])
                        yield
                    yield ("unit", pair, h2, qch, pbt)

                def emit_pv_norm(pair, h2, qch, pbt, o_norm):
                    """PV + softmax-normalize for a unit whose probs are done."""
                    vg = vg_tiles[pair // 2]
                    hl = (pair * 2 + h2) % 4
                    pvt = pvp.tile([128, 4, DH + 1], F32, tag="pvt")
                    for qt in range(4):
                        for kt in range(TS // 128):
                            nc.tensor.matmul(
                                pvt[:, qt, :],
                                pbt[kt // 2][:, kt % 2,
                                             qt * 128:(qt + 1) * 128],
                                vg[:, kt, hl, :],
                                start=(kt == 0), stop=(kt == TS // 128 - 1))
                    rec = asm.tile([128, 4], F32, tag="rec")
                    nc.vector.reciprocal(out=rec[:], in_=pvt[:, :, DH])
                    nc.vector.tensor_tensor(
                        out=o_norm[:, qch * 4:qch * 4 + 4, h2, :],
                        in0=pvt[:, :, 0:DH], in1=_bcast0(rec[:], DH),
                        op=mybir.AluOpType.mult)

                def emit_oT(pair, o_norm):
                    """Transpose pair's o chunk -> oT hi/lo (c-chunk = pair)."""
                    for ch in range(2):
                        pt = pso.tile([128, 512], F32, tag="pso")
                        for i in range(4):
                            qt = 4 * ch + i
                            nc.tensor.matmul(
                                pt[:, i * 128:(i + 1) * 128],
                                o_norm[:, qt, :, :].rearrange("p h d -> p (h d)"),
                                ident_b[:], start=True, stop=True)
                        nc.vector.tensor_copy(out=oT_hi[ch][:, pair, :], in_=pt[:])
                        nc.vector.scalar_tensor_tensor(
                            out=oT_lo[ch][:, pair, :],
                            in0=pt[:], scalar=1.0,
                            in1=oT_hi[ch][:, pair, :],
                            op0=mybir.AluOpType.mult,
                            op1=mybir.AluOpType.subtract)

                def drain(gen, n=None):
                    k = 0
                    for _ in gen:
                        k += 1
                        if n is not None and k >= n:
                            return True
                    return False

                def gen_chain(g):
                    yield from qkv_gen(g)
                    yield from v_gen(g)

                drain(gen_chain(0))
                cur = [None]
                nqk = [1]

                def pull_qk(pair, n):
                    for _ in range(n):
                        if cur[0] is None and nqk[0] < 4 and nqk[0] <= pair // 2 + 1:
                            cur[0] = gen_chain(nqk[0])
                            nqk[0] += 1
                        if cur[0] is None:
                            return
                        if not drain(cur[0], 1):
                            cur[0] = None

                ycnt = [0]
                o_norms = {}
                pending = [None]  # (pair, h2, qch, pbt)

                def flush_pending():
                    if pending[0] is not None:
                        p_, h2_, qch_, pbt_ = pending[0]
                        emit_pv_norm(p_, h2_, qch_, pbt_, o_norms[p_])
                        pending[0] = None
                        if h2_ == 1 and qch_ == TQ // 512 - 1:
                            emit_oT(p_, o_norms.pop(p_))

                for pair in range(H // 2):
                    while pair not in qk_tiles or pair // 2 not in vg_tiles:
                        pull_qk(pair, 1)
                    o_norms[pair] = onp.tile([128, TQ // 128, 2, DH], BF16,
                                             tag="o_norm", name=f"o_norm{pair}")
                    for h2 in range(2):
                        for qch in range(TQ // 512):
                            for tok in emit_scores_exp(pair, h2, qch):
                                if isinstance(tok, tuple):
                                    flush_pending()
                                    pending[0] = (pair, h2, qch, tok[4])
                                else:
                                    ycnt[0] += 1
                                    if ycnt[0] % 2 == 0:
                                        pull_qk(pair, 1)
                flush_pending()

            # ============ Stage D: oT split, Wo (3-term fp8), residual, LN2 ====
            with contextlib.ExitStack() as dstk:
                x2p = dstk.enter_context(tc.tile_pool(name="x2p", bufs=1))
                xn2p = dstk.enter_context(tc.tile_pool(name="xn2p", bufs=2))
                x2 = x2p.tile([128, TQ // 128, C], F32R, tag="x2")
                xn2_hi = [xn2p.tile([128, NKO, 512], E4, tag="xn2hi",
                                    name=f"xn2hi{i}") for i in range(2)]
                xn2_lo = [xn2p.tile([128, NKO, 512], E5, tag="xn2lo",
                                    name=f"xn2lo{i}") for i in range(2)]

                pst2 = dstk.enter_context(tc.tile_pool(name="pst2", bufs=3,
                                                       space="PSUM"))
                psE = dstk.enter_context(tc.tile_pool(name="psE", bufs=3,
                                                      space="PSUM"))
                with contextlib.ExitStack() as dd:
                    aop = dd.enter_context(tc.tile_pool(name="aop", bufs=8))
                    wop = dd.enter_context(tc.tile_pool(name="wop", bufs=1))
                    workD = dd.enter_context(tc.tile_pool(name="workD", bufs=3))
                    wo_hi = wop.tile([128, NKO, C], E4, tag="wohi")
                    wo_lo = wop.tile([128, NKO, C], E5, tag="wolo")
                    nc.sync.dma_start(out=wo_hi[:],
                                      in_=woh_d.rearrange("(o p) f -> p o f", p=128))
                    nc.sync.dma_start(out=wo_lo[:],
                                      in_=wol_d.rearrange("(o p) f -> p o f", p=128))
                    aoT = [aop.tile([128, TQ], BF16, tag="aoT",
                                    name=f"aoT{i}") for i in range(NKO)]

                    # Wo: aoT[f, t] = sum_c oT[c, t] * wo[c, f]  (3-term fp8)
                    for f in range(NKO):
                        for ch in range(TQ // 512):
                            pw = psE.tile([128, 512], F32, tag="psE")
                            for kop in range(NKO // 2):
                                ksl = slice(2 * kop, 2 * kop + 2)
                                fsl = slice(f * 128, (f + 1) * 128)
                                nc.tensor.matmul(pw[:], wo_hi[:, ksl, fsl],
                                                 oT_hi[ch][:, ksl, :], perf_mode=DR,
                                                 start=(kop == 0), stop=False)
                                nc.tensor.matmul(pw[:], wo_lo[:, ksl, fsl],
                                                 oT_hi[ch][:, ksl, :], perf_mode=DR,
                                                 start=False, stop=False)
                                nc.tensor.matmul(pw[:], wo_hi[:, ksl, fsl],
                                                 oT_lo[ch][:, ksl, :], perf_mode=DR,
                                                 start=False, stop=(kop == NKO // 2 - 1))
                            nc.scalar.activation(
                                out=aoT[f][:, ch * 512:(ch + 1) * 512], in_=pw[:],
                                func=mybir.ActivationFunctionType.Identity,
                                bias=bo_s[:, f:f + 1], scale=1.0 / WSCALE)

                    # aoT back to token-major + residual -> x2; LN2 -> xn2 hi/lo
                    def finishD(t, mv, rstd):
                        xn2_r = workD.tile([128, C], F32R, tag="xn2_r")
                        _ln_stats_b(nc, mv, rstd, x2[:, t, :], xn2_r[:],
                                    pool=(t % 2 == 0))
                        for cg in range(2):
                            pt = pst2.tile([128, 4, 128], F32R, tag="pst2")
                            for i in range(4):
                                c = 4 * cg + i
                                nc.tensor.transpose(
                                    pt[:, i, :],
                                    xn2_r[:, c * 128:(c + 1) * 128], ident_r[:])
                            xsl = (slice(4 * cg, 4 * cg + 4),
                                   slice((t % 4) * 128, (t % 4 + 1) * 128))
                            nc.scalar.activation(
                                out=xn2_hi[t // 4][:, xsl[0], xsl[1]],
                                in_=pt[:], func=mybir.ActivationFunctionType.Copy,
                                bias=0.0, scale=1.0)
                            nc.vector.scalar_tensor_tensor(
                                out=xn2_lo[t // 4][:, xsl[0], xsl[1]],
                                in0=pt[:], scalar=1.0,
                                in1=xn2_hi[t // 4][:, xsl[0], xsl[1]],
                                op0=mybir.AluOpType.mult,
                                op1=mybir.AluOpType.subtract)

                    prevD = None
                    for t in range(TQ // 128):
                        x_t = workD.tile([128, C], F32R, tag="x_t")
                        nc.sync.dma_start(out=x_t[:],
                                          in_=x_d[t * 128:(t + 1) * 128, :].bitcast(F32R))
                        for cg in range(2):
                            pt = pst2.tile([128, 4, 128], F32, tag="pst2")
                            nc.tensor.matmul(
                                pt[:].rearrange("p a b -> p (a b)"), ident_r[:],
                                x_t[:, cg * 512:(cg + 1) * 512],
                                start=True, stop=False, skip_group_check=True)
                            for i in range(4):
                                c = 4 * cg + i
                                nc.tensor.matmul(
                                    pt[:, i, :],
                                    aoT[c][:, t * 128:(t + 1) * 128],
                                    ident_b[:], start=False, stop=(i == 3),
                                    skip_group_check=True)
                            nc.scalar.activation(
                                out=x2[:, t, cg * 512:(cg + 1) * 512],
                                in_=pt[:].rearrange("p a b -> p (a b)"),
                                func=mybir.ActivationFunctionType.Copy,
                                bias=0.0, scale=1.0)
                        mv, rstd = _ln_stats_a(nc, stats, x2[:, t, :], eps_t)
                        if prevD is not None:
                            finishD(*prevD)
                        prevD = (t, mv, rstd)
                    finishD(*prevD)

                # ============ Stage E: FFN up (W1, relu) 3-term fp8 ============
                h1p = dstk.enter_context(tc.tile_pool(name="h1p", bufs=1))
                h1_hi = h1p.tile([128, DFF // 128, TQ], E4, tag="h1hi")
                h1_lo = h1p.tile([128, DFF // 128, TQ], E5, tag="h1lo")
                w1h_r = w1h_d.rearrange("(o p) f -> p o f", p=128)
                w1l_r = w1l_d.rearrange("(o p) f -> p o f", p=128)
                with tc.tile_pool(name="w1p", bufs=2) as w1p:
                    for blk in range(DFF // 512):
                        w1th = w1p.tile([128, NKO, 512], E4, tag="w1th")
                        w1tl = w1p.tile([128, NKO, 512], E5, tag="w1tl")
                        nc.sync.dma_start(out=w1th[:],
                                          in_=w1h_r[:, :, blk * 512:(blk + 1) * 512])
                        nc.sync.dma_start(out=w1tl[:],
                                          in_=w1l_r[:, :, blk * 512:(blk + 1) * 512])
                        for ch in range(TQ // 512):
                            csl = slice(ch * 512, (ch + 1) * 512)
                            for fs in range(4):
                                f = blk * 4 + fs
                                fsl = slice(fs * 128, (fs + 1) * 128)
                                ph = psE.tile([128, 512], F32, tag="psE")
                                for kop in range(NKO // 2):
                                    ksl = slice(2 * kop, 2 * kop + 2)
                                    nc.tensor.matmul(ph[:], w1th[:, ksl, fsl],
                                                     xn2_hi[ch][:, ksl, :], perf_mode=DR,
                                                     start=(kop == 0), stop=False)
                                    nc.tensor.matmul(ph[:], w1tl[:, ksl, fsl],
                                                     xn2_hi[ch][:, ksl, :], perf_mode=DR,
                                                     start=False, stop=False)
                                    nc.tensor.matmul(ph[:], w1th[:, ksl, fsl],
                                                     xn2_lo[ch][:, ksl, :], perf_mode=DR,
                                                     start=False,
                                                     stop=(kop == NKO // 2 - 1))
                                nc.scalar.activation(
                                    out=h1_hi[:, f, csl], in_=ph[:],
                                    func=mybir.ActivationFunctionType.Relu,
                                    bias=b1_s[:, f:f + 1], scale=1.0)
                                nc.vector.scalar_tensor_tensor(
                                    out=h1_lo[:, f, csl], in0=ph[:], scalar=0.0,
                                    in1=h1_hi[:, f, csl],
                                    op0=mybir.AluOpType.max,
                                    op1=mybir.AluOpType.subtract)

                # ============ Stage F: FFN down (W2) 3-term fp8 + residual ======
                fp = dstk.enter_context(tc.tile_pool(name="fp", bufs=8))
                ffnT = [fp.tile([128, TQ], BF16, tag="ffnT",
                                name=f"ffnT{i}") for i in range(NKO)]
                w2h_r = w2h_d.rearrange("(o p) f -> p o f", p=128)
                w2l_r = w2l_d.rearrange("(o p) f -> p o f", p=128)
                with tc.tile_pool(name="w2p", bufs=2) as w2p:
                    for f in range(NKO):
                        fsl = slice(f * 128, (f + 1) * 128)
                        w2th = w2p.tile([128, DFF // 128, 128], E4, tag="w2th")
                        w2tl = w2p.tile([128, DFF // 128, 128], E5, tag="w2tl")
                        nc.sync.dma_start(out=w2th[:], in_=w2h_r[:, :, fsl])
                        nc.sync.dma_start(out=w2tl[:], in_=w2l_r[:, :, fsl])
                        for ch in range(TQ // 512):
                            csl = slice(ch * 512, (ch + 1) * 512)
                            po2 = psE.tile([128, 512], F32, tag="psE")
                            for kop in range(DFF // 256):
                                ksl = slice(2 * kop, 2 * kop + 2)
                                nc.tensor.matmul(po2[:], w2th[:, ksl, :],
                                                 h1_hi[:, ksl, csl], perf_mode=DR,
                                                 start=(kop == 0), stop=False)
                                nc.tensor.matmul(po2[:], w2tl[:, ksl, :],
                                                 h1_hi[:, ksl, csl], perf_mode=DR,
                                                 start=False, stop=False)
                                nc.tensor.matmul(po2[:], w2th[:, ksl, :],
                                                 h1_lo[:, ksl, csl], perf_mode=DR,
                                                 start=False,
                                                 stop=(kop == DFF // 256 - 1))
                            nc.scalar.activation(
                                out=ffnT[f][:, csl], in_=po2[:],
                                func=mybir.ActivationFunctionType.Identity,
                                bias=b2_s[:, f:f + 1],
                                scale=1.0 / (WSCALE * WSCALE))
                with tc.tile_pool(name="workF", bufs=2) as workF:
                    for t in range(TQ // 128):
                        out_t = workF.tile([128, C], F32, tag="out_t")
                        for cg in range(2):
                            pt = pst2.tile([128, 4, 128], F32, tag="pst2")
                            nc.tensor.matmul(
                                pt[:].rearrange("p a b -> p (a b)"), ident_r[:],
                                x2[:, t, cg * 512:(cg + 1) * 512],
                                start=True, stop=False, skip_group_check=True)
                            for i in range(4):
                                c = 4 * cg + i
                                nc.tensor.matmul(
                                    pt[:, i, :],
                                    ffnT[c][:, t * 128:(t + 1) * 128],
                                    ident_b[:], start=False, stop=(i == 3),
                                    skip_group_check=True)
                            nc.scalar.activation(
                                out=out_t[:, cg * 512:(cg + 1) * 512],
                                in_=pt[:].rearrange("p a b -> p (a b)"),
                                func=mybir.ActivationFunctionType.Copy,
                                bias=0.0, scale=1.0)
                        nc.sync.dma_start(out=out_d[t * 128:(t + 1) * 128, :],
                                          in_=out_t[:])

    nc.finalize()
    _legalize_sem_waits(nc)
    return nc


_NC_CACHE = None


def _get_nc():
    global _NC_CACHE
    if _NC_CACHE is None:
        _NC_CACHE = _build_nc()
    return _NC_CACHE


def _split_w(w, scale=WSCALE):
    ws = np.asarray(w, np.float32) * scale
    hi = ws.astype(ml_dtypes.float8_e4m3)
    lo = (ws - hi.astype(np.float32)).astype(ml_dtypes.float8_e5m2)
    return np.ascontiguousarray(hi), np.ascontiguousarray(lo)


def _shard_inputs(inputs):
    x = np.asarray(inputs["x"], np.float32)
    ln1_g = np.asarray(inputs["ln1_g"], np.float32).reshape(C)
    ln1_b = np.asarray(inputs["ln1_b"], np.float32).reshape(C)
    ln2_g = np.asarray(inputs["ln2_g"], np.float32).reshape(C)
    ln2_b = np.asarray(inputs["ln2_b"], np.float32).reshape(C)
    wq = np.ascontiguousarray(
        np.transpose(np.asarray(inputs["Wq"], np.float32), (1, 0, 2)).reshape(C, C))
    wk = np.ascontiguousarray(
        np.transpose(np.asarray(inputs["Wk"], np.float32), (1, 0, 2)).reshape(C, C))
    wv = np.ascontiguousarray(
        np.transpose(np.asarray(inputs["Wv"], np.float32), (1, 0, 2)).reshape(C, C))
    wo = np.asarray(inputs["Wo"], np.float32)
    w1 = np.asarray(inputs["W1"], np.float32)
    w2 = np.asarray(inputs["W2"], np.float32)

    # fold LN affine into the consuming weights/biases
    bq = np.asarray(inputs["bq"], np.float32).reshape(C) + ln1_b @ wq
    bk = np.asarray(inputs["bk"], np.float32).reshape(C) + ln1_b @ wk
    bv = np.asarray(inputs["bv"], np.float32).reshape(C) + ln1_b @ wv
    wq = np.ascontiguousarray(ln1_g[:, None] * wq)
    wk = np.ascontiguousarray(ln1_g[:, None] * wk)
    wv = np.ascontiguousarray(ln1_g[:, None] * wv)
    b1 = WSCALE * (np.asarray(inputs["b1"], np.float32).reshape(DFF) + ln2_b @ w1)
    assert np.abs(b1).max() == 0.0, "nonzero effective W1 bias unsupported by lo-split"
    w1g = ln2_g[:, None] * w1

    woh, wol = _split_w(wo)
    w1h, w1l = _split_w(w1g)
    w2h, w2l = _split_w(w2)

    shared = {
        "wq": wq, "wk": wk, "wv": wv,
        "woh": woh, "wol": wol, "w1h": w1h, "w1l": w1l, "w2h": w2h, "w2l": w2l,
        "bq": bq, "bk": bk, "bv": bv.astype(ml_dtypes.bfloat16),
        "bo": np.asarray(inputs["bo"], np.float32).reshape(C),
        "b1": b1,
        "b2": np.asarray(inputs["b2"], np.float32).reshape(C),
    }
    in_maps = []
    for c in range(N_CORES):
        b, half = c // 2, c % 2
        own = x[b, half * TQ:(half + 1) * TQ]
        other = x[b, (1 - half) * TQ:(2 - half) * TQ]
        x_perm = np.ascontiguousarray(np.concatenate([own, other], axis=0))
        in_maps.append(dict(shared, x=x_perm))
    return in_maps


def _run(inputs, **spmd_kwargs):
    nc = _get_nc()
    in_maps = _shard_inputs(inputs)
    res = run_bass_kernel_spmd(nc, in_maps, core_ids=list(range(N_CORES)), **spmd_kwargs)
    out = np.empty((B, T, C), np.float32)
    for c in range(N_CORES):
        b, half = c // 2, c % 2
        out[b, half * TQ:(half + 1) * TQ] = res.results[c]["out"]
    return out, res


def kernel(**inputs) -> np.ndarray:
    out, _ = _run(inputs)
    return out


# revision 59
# speedup vs baseline: 1.0200x; 1.0038x over previous
"""Trainium2 Bass kernel for a pre-norm transformer block (MHSA + FFN).

Sharding: 8 cores, data parallel over (batch, seq-half). Core c handles
batch c//2, sequence half c%2. Inputs are permuted so each core's own
1024 tokens come first; attention K/V run over all 2048 tokens of the
batch (softmax is permutation invariant).

Numerics: Q/K projections and scores in f32r (softmax logits are
~N(0,26^2) — fp8 there flips argmaxes). Softmax probs in bf16 with a
constant exp shift; PV runs probs-stationary so only the 65-wide
(dh+denominator) V operand streams. Wo/W1/W2 run as 3-term compensated
fp8 DoubleRow (hi=e4m3, lo=e5m2, x@W ~= xh@Wh + xh@Wl + xl@Wh), with
weights pre-scaled x32 and split host-side. LayerNorm gains/biases are
folded into the downstream weights/biases host-side, so on-chip LN is
pure z-normalization and the hi/lo activation splits are single
scalar_tensor_tensor ops.
"""
import contextlib

import numpy as np
import ml_dtypes

import concourse.bass as bass
import concourse.tile as tile
import concourse.mybir as mybir
from concourse.bass_utils import run_bass_kernel_spmd
from concourse.masks import make_identity

B, T, C = 4, 2048, 1024
H, DH = 16, 64
DFF = 4 * C
N_CORES = 8
TQ = T // 2          # tokens owned per core
TS = T               # key/value tokens per core
NKO = C // 128       # 8 contraction tiles for C
F32R = mybir.dt.float32r
F32 = mybir.dt.float32
BF16 = mybir.dt.bfloat16
E4 = mybir.dt.float8e4
E5 = mybir.dt.float8e5
EXP_BIAS = -128.0
EPS = 1e-5
WSCALE = 32.0
DR = mybir.MatmulPerfMode.DoubleRow

# ---------------------------------------------------------------------------
# Compat: this walrus build accepts at most 1 sem-wait per regular
# instruction (2 per InstEventSemaphore). bacc misses some tile-generated
# instructions, so split waits ourselves after finalize.
_ev_counter = [0]


def _legalize_sem_waits(nc):
    for func in nc.m.functions:
        for bb in func.blocks:
            new = []
            changed = False
            for inst in bb.instructions:
                si = inst.sync_info
                cap = 2 if isinstance(inst, mybir.InstEventSemaphore) else 1
                if si is not None and len(si.on_wait) > cap:
                    waits = list(si.on_wait)
                    for i in range(cap, len(waits), 2):
                        _ev_counter[0] += 1
                        e = mybir.InstEventSemaphore(
                            name=f"EVSPLIT-{_ev_counter[0]}", ins=[], outs=[])
                        e.engine = inst.engine
                        e.sync_info = mybir.SyncInfo(
                            on_wait=waits[i:i + 2], on_update=[])
                        new.append(e)
                    inst.sync_info = mybir.SyncInfo(
                        on_wait=waits[:cap], on_update=list(si.on_update))
                    changed = True
                new.append(inst)
            if changed:
                bb.instructions = new


# ---------------------------------------------------------------------------

def _ln_stats_a(nc, stats, x_ap, eps_t):
    """bn stats + sqrt(var+eps) for x_ap [128, C]; returns (mv, rstd)."""
    st = stats.tile([128, 2, 6], F32, tag="bnstats")
    mv = stats.tile([128, 2], F32, tag="bnaggr")
    xg = x_ap.rearrange("p (s d) -> p s d", s=2)
    for s in range(2):
        nc.vector.bn_stats(out=st[:, s, :], in_=xg[:, s, :])
    nc.vector.bn_aggr(out=mv[:], in_=st[:])
    rstd = stats.tile([128, 1], F32, tag="rstd")
    nc.scalar.activation(out=rstd[:], in_=mv[:, 1:2],
                         func=mybir.ActivationFunctionType.Sqrt,
                         bias=eps_t[:], scale=1.0)
    return mv, rstd


def _ln_stats_b(nc, mv, rstd, x_ap, out_ap, pool=False):
    """finish z-normalize: recip + (x - mu) * rstd."""
    nc.vector.reciprocal(out=rstd[:], in_=rstd[:])
    eng = nc.gpsimd if pool else nc.vector
    eng.tensor_scalar(out=out_ap, in0=x_ap,
                      scalar1=mv[:, 0:1], scalar2=rstd[:],
                      op0=mybir.AluOpType.subtract,
                      op1=mybir.AluOpType.mult)


def _bcast0(ap, free):
    """Broadcast a [128, n] AP along a new stride-0 free dim of size `free`."""
    return bass.AP(tensor=ap.tensor, offset=ap.offset,
                   ap=[list(d) for d in ap.ap] + [[0, free]])


def _build_nc():
    nc = bass.Bass()

    # ---- I/O ----
    x_d = nc.dram_tensor("x", [T, C], F32, kind="ExternalInput")
    wq_d = nc.dram_tensor("wq", [C, C], F32R, kind="ExternalInput")
    wk_d = nc.dram_tensor("wk", [C, C], F32R, kind="ExternalInput")
    wv_d = nc.dram_tensor("wv", [C, C], F32R, kind="ExternalInput")
    woh_d = nc.dram_tensor("woh", [C, C], E4, kind="ExternalInput")
    wol_d = nc.dram_tensor("wol", [C, C], E5, kind="ExternalInput")
    w1h_d = nc.dram_tensor("w1h", [C, DFF], E4, kind="ExternalInput")
    w1l_d = nc.dram_tensor("w1l", [C, DFF], E5, kind="ExternalInput")
    w2h_d = nc.dram_tensor("w2h", [DFF, C], E4, kind="ExternalInput")
    w2l_d = nc.dram_tensor("w2l", [DFF, C], E5, kind="ExternalInput")
    bq_d = nc.dram_tensor("bq", [C], F32, kind="ExternalInput")
    bk_d = nc.dram_tensor("bk", [C], F32, kind="ExternalInput")
    bv_d = nc.dram_tensor("bv", [C], BF16, kind="ExternalInput")
    bo_d = nc.dram_tensor("bo", [C], F32, kind="ExternalInput")
    b1_d = nc.dram_tensor("b1", [DFF], F32, kind="ExternalInput")
    b2_d = nc.dram_tensor("b2", [C], F32, kind="ExternalInput")
    out_d = nc.dram_tensor("out", [TQ, C], F32, kind="ExternalOutput")

    wq_r = wq_d.rearrange("(o p) f -> p o f", p=128)
    wk_r = wk_d.rearrange("(o p) f -> p o f", p=128)
    wv_r = wv_d.rearrange("(o p) f -> p o f", p=128)

    with tile.TileContext(nc) as tc:
        with contextlib.ExitStack() as top:
            consts = top.enter_context(tc.tile_pool(name="consts", bufs=1))
            stats = top.enter_context(tc.tile_pool(name="stats", bufs=8))
            ps = top.enter_context(tc.tile_pool(name="ps", bufs=2, space="PSUM"))

            ident_b = consts.tile([128, 128], BF16, tag="identb")
            make_identity(nc, ident_b)
            ident_r = consts.tile([128, 128], F32R, tag="identr")
            nc.vector.tensor_copy(out=ident_r[:], in_=ident_b[:])
            ebias = consts.tile([128, 1], F32, tag="ebias")
            nc.vector.memset(ebias[:], EXP_BIAS)
            eps_t = consts.tile([128, 1], F32, tag="eps")
            nc.vector.memset(eps_t[:], EPS)
            bq_s = consts.tile([128, NKO], F32, tag="bq")
            bk_s = consts.tile([128, NKO], F32, tag="bk")
            bo_s = consts.tile([128, NKO], F32, tag="bo")
            b2_s = consts.tile([128, NKO], F32, tag="b2")
            b1_s = consts.tile([128, DFF // 128], F32, tag="b1")
            for dst, src in ((bq_s, bq_d), (bk_s, bk_d), (bo_s, bo_d),
                             (b2_s, b2_d), (b1_s, b1_d)):
                nc.sync.dma_start(out=dst[:], in_=src.rearrange("(o p) -> p o", p=128))
            bv_r = consts.tile([128, C], BF16, tag="bvr")
            nc.gpsimd.dma_start(
                out=bv_r[:],
                in_=bass.AP(tensor=bv_d[:].tensor, offset=bv_d[:].offset,
                            ap=[[0, 128]] + [list(d) for d in bv_d[:].ap]))

            # ============ Stages A-C: LN1, QKV, attention ============
            with contextlib.ExitStack() as abc:
                xnp = abc.enter_context(tc.tile_pool(name="xnp", bufs=4))
                xnT_blks = [xnp.tile([128, NKO, 512], F32R, tag="xnT",
                                     name=f"xnT{i}") for i in range(4)]

                # ---- Stage A: LN1 (z-norm only) + transpose -> xnT ----
                with tc.tile_pool(name="workA", bufs=6) as workA, \
                     tc.tile_pool(name="pstA", bufs=2, space="PSUM") as pstA:
                    def finishA(t, x_t, mv, rstd):
                        xn_r = workA.tile([128, C], F32R, tag="xn_r")
                        _ln_stats_b(nc, mv, rstd, x_t[:], xn_r[:],
                                    pool=(t % 2 == 0))
                        for cg in range(2):
                            pt = pstA.tile([128, 4, 128], F32R, tag="pstA")
                            for i in range(4):
                                nc.tensor.transpose(
                                    pt[:, i, :],
                                    xn_r[:, (4 * cg + i) * 128:(4 * cg + i + 1) * 128],
                                    ident_r[:])
                            nc.scalar.activation(
                                out=xnT_blks[t // 4][:, 4 * cg:4 * cg + 4,
                                                     (t % 4) * 128:(t % 4 + 1) * 128],
                                in_=pt[:],
                                func=mybir.ActivationFunctionType.Copy,
                                bias=0.0, scale=1.0)

                    prevA = None
                    for t in range(T // 128):
                        x_t = workA.tile([128, C], F32, tag="x_t")
                        nc.sync.dma_start(out=x_t[:], in_=x_d[t * 128:(t + 1) * 128, :])
                        mv, rstd = _ln_stats_a(nc, stats, x_t[:], eps_t)
                        if prevA is not None:
                            finishA(*prevA)
                        prevA = (t, x_t, mv, rstd)
                    finishA(*prevA)

                # ---- Stages B+C interleaved ----
                wgp = abc.enter_context(tc.tile_pool(name="wgp", bufs=1))
                qkp = abc.enter_context(tc.tile_pool(name="qkp", bufs=2))
                vgp = abc.enter_context(tc.tile_pool(name="vgp", bufs=2))
                prb = abc.enter_context(tc.tile_pool(name="probs", bufs=13))
                onp = abc.enter_context(tc.tile_pool(name="onp", bufs=2))
                otp = abc.enter_context(tc.tile_pool(name="otp", bufs=2))
                pvp = abc.enter_context(tc.tile_pool(name="pvp", bufs=1, space="PSUM"))
                pso = abc.enter_context(tc.tile_pool(name="pso", bufs=1, space="PSUM"))
                ps2 = abc.enter_context(tc.tile_pool(name="ps2", bufs=2, space="PSUM"))
                asm = abc.enter_context(tc.tile_pool(name="att_sm", bufs=3))
                schp = abc.enter_context(tc.tile_pool(name="schp", bufs=2))

                oT_hi = [otp.tile([128, NKO, 512], E4, tag="oThi",
                                  name=f"oThi{i}") for i in range(2)]
                oT_lo = [otp.tile([128, NKO, 512], E5, tag="oTlo",
                                  name=f"oTlo{i}") for i in range(2)]

                qk_tiles = {}
                vg_tiles = {}

                def qkv_gen(g):
                    """Q/K for pairs 2g, 2g+1. Yields after each psum group."""
                    wqt = wgp.tile([128, NKO, 256], F32R, tag="wqt")
                    wkt = wgp.tile([128, NKO, 256], F32R, tag="wkt")
                    nc.sync.dma_start(out=wqt[:], in_=wq_r[:, :, g * 256:(g + 1) * 256])
                    nc.sync.dma_start(out=wkt[:], in_=wk_r[:, :, g * 256:(g + 1) * 256])
                    for i, f in enumerate((2 * g, 2 * g + 1)):
                        qps = [qkp.tile([128, 512], F32R, tag=f"qp{i}c{ch}",
                                        name=f"qp{f}c{ch}")
                               for ch in range(TQ // 512)]
                        kps = [qkp.tile([128, 512], F32R, tag=f"kp{i}c{ch}",
                                        name=f"kp{f}c{ch}")
                               for ch in range(TS // 512)]
                        qk_tiles[2 * g + i] = (qps, kps)
                        for ch in range(TQ // 512):
                            pq = ps.tile([128, 512], F32, tag="ps")
                            for ko in range(NKO):
                                nc.tensor.matmul(pq[:], wqt[:, ko, i * 128:(i + 1) * 128],
                                                 xnT_blks[ch][:, ko, :],
                                                 start=(ko == 0), stop=(ko == NKO - 1))
                            nc.vector.tensor_scalar(
                                out=qps[ch][:], in0=pq[:],
                                scalar1=bq_s[:, f:f + 1], scalar2=None,
                                op0=mybir.AluOpType.add)
                            yield
                        for ch in range(TS // 512):
                            pk = ps.tile([128, 512], F32, tag="ps")
                            for ko in range(NKO):
                                nc.tensor.matmul(pk[:], wkt[:, ko, i * 128:(i + 1) * 128],
                                                 xnT_blks[ch][:, ko, :],
                                                 start=(ko == 0), stop=(ko == NKO - 1))
                            nc.vector.tensor_scalar(
                                out=kps[ch][:], in0=pk[:],
                                scalar1=bk_s[:, f:f + 1], scalar2=None,
                                op0=mybir.AluOpType.add)
                            yield

                def v_gen(g):
                    """V for heads 4g..4g+3 -> vg tile [128, 16, 4, 65] bf16."""
                    wvt = wgp.tile([128, NKO, 256], F32R, tag="wvt")
                    nc.sync.dma_start(out=wvt[:], in_=wv_r[:, :, g * 256:(g + 1) * 256])
                    vg = vgp.tile([128, TS // 128, 4, 65], BF16, tag="vg")
                    vg_tiles[g] = vg
                    nc.vector.memset(vg[:, :, :, DH:DH + 1], 1.0)
                    for to in range(TS // 128):
                        pv = ps.tile([128, 512], F32, tag="ps")
                        for ko in range(NKO):
                            nc.tensor.matmul(pv[0:128, 0:256],
                                             xnT_blks[to // 4][:, ko,
                                                 (to % 4) * 128:(to % 4 + 1) * 128],
                                             wvt[:, ko, :],
                                             start=(ko == 0), stop=(ko == NKO - 1))
                        nc.vector.tensor_tensor(
                            out=vg[:, to, :, 0:DH],
                            in0=pv[:, 0:256].rearrange("p (h d) -> p h d", d=DH),
                            in1=bv_r[:, g * 256:(g + 1) * 256].rearrange(
                                "p (h d) -> p h d", d=DH),
                            op=mybir.AluOpType.add)
                        yield

                def emit_scores_exp(pair, h2, qch):
                    """Scores + exp for one (head, qch) unit. Yields per ktg."""
                    qps, kps = qk_tiles[pair]
                    base = h2 * 64
                    pbt = [prb.tile([128, 2, 512], BF16, tag="probsT",
                                    name=f"pb{kg}")
                           for kg in range(TS // 256)]
                    for ktg in range(TS // 256):
                        psc = ps2.tile([128, 2, 512], F32, tag="psc")
                        for j in range(2):
                            kt = 2 * ktg + j
                            nc.tensor.matmul(
                                psc[:, j, :],
                                kps[kt // 4][base:base + DH,
                                             (kt % 4) * 128:(kt % 4 + 1) * 128],
                                qps[qch][base:base + DH, :],
                                start=True, stop=True)
                        if pair >= 5 and ktg in (1, 4):
                            # Schraudolph exp2 on DVE: bits = y*K1+K2,
                            # bitcast to f32, clamp negatives to 0
                            for j in range(2):
                                sch = schp.tile([128, 512],
                                                mybir.dt.int32, tag="sch")
                                nc.vector.tensor_scalar(
                                    out=sch[:], in0=psc[:, j, :],
                                    scalar1=96817625.34,
                                    scalar2=-484236300.5,
                                    op0=mybir.AluOpType.mult,
                                    op1=mybir.AluOpType.add)
                                nc.vector.tensor_scalar(
                                    out=pbt[ktg][:, j, :],
                                    in0=sch[:].bitcast(F32), scalar1=0.0,
                                    scalar2=None, op0=mybir.AluOpType.max)
                        else:
                            nc.scalar.activation(
                                out=pbt[ktg][:], in_=psc[:],
                                func=mybir.ActivationFunctionType.Exp,
                                scale=8.0, bias=ebias[:])
                        yield
                    yield ("unit", pair, h2, qch, pbt)

                def emit_pv_norm(pair, h2, qch, pbt, o_norm):
                    """PV + softmax-normalize for a unit whose probs are done."""
                    vg = vg_tiles[pair // 2]
                    hl = (pair * 2 + h2) % 4
                    pvt = pvp.tile([128, 4, DH + 1], F32, tag="pvt")
                    for qt in range(4):
                        for kt in range(TS // 128):
                            nc.tensor.matmul(
                                pvt[:, qt, :],
                                pbt[kt // 2][:, kt % 2,
                                             qt * 128:(qt + 1) * 128],
                                vg[:, kt, hl, :],
                                start=(kt == 0), stop=(kt == TS // 128 - 1))
                    rec = asm.tile([128, 4], F32, tag="rec")
                    nc.vector.reciprocal(out=rec[:], in_=pvt[:, :, DH])
                    nc.vector.tensor_tensor(
                        out=o_norm[:, qch * 4:qch * 4 + 4, h2, :],
                        in0=pvt[:, :, 0:DH], in1=_bcast0(rec[:], DH),
                        op=mybir.AluOpType.mult)

                def emit_oT(pair, o_norm):
                    """Transpose pair's o chunk -> oT hi/lo (c-chunk = pair)."""
                    for ch in range(2):
                        pt = pso.tile([128, 512], F32, tag="pso")
                        for i in range(4):
                            qt = 4 * ch + i
                            nc.tensor.matmul(
                                pt[:, i * 128:(i + 1) * 128],
                                o_norm[:, qt, :, :].rearrange("p h d -> p (h d)"),
                                ident_b[:], start=True, stop=True)
                        nc.vector.tensor_copy(out=oT_hi[ch][:, pair, :], in_=pt[:])
                        nc.vector.scalar_tensor_tensor(
                            out=oT_lo[ch][:, pair, :],
                            in0=pt[:], scalar=1.0,
                            in1=oT_hi[ch][:, pair, :],
                            op0=mybir.AluOpType.mult,
                            op1=mybir.AluOpType.subtract)

                def drain(gen, n=None):
                    k = 0
                    for _ in gen:
                        k += 1
                        if n is not None and k >= n:
                            return True
                    return False

                def gen_chain(g):
                    yield from qkv_gen(g)
                    yield from v_gen(g)

                drain(gen_chain(0))
                cur = [None]
                nqk = [1]

                def pull_qk(pair, n):
                    for _ in range(n):
                        if cur[0] is None and nqk[0] < 4 and nqk[0] <= pair // 2 + 1:
                            cur[0] = gen_chain(nqk[0])
                            nqk[0] += 1
                        if cur[0] is None:
                            return
                        if not drain(cur[0], 1):
                            cur[0] = None

                ycnt = [0]
                o_norms = {}
                pending = [None]  # (pair, h2, qch, pbt)

                def flush_pending():
                    if pending[0] is not None:
                        p_, h2_, qch_, pbt_ = pending[0]
                        emit_pv_norm(p_, h2_, qch_, pbt_, o_norms[p_])
                        pending[0] = None
                        if h2_ == 1 and qch_ == TQ // 512 - 1:
                            emit_oT(p_, o_norms.pop(p_))

                for pair in range(H // 2):
                    while pair not in qk_tiles or pair // 2 not in vg_tiles:
                        pull_qk(pair, 1)
                    o_norms[pair] = onp.tile([128, TQ // 128, 2, DH], BF16,
                                             tag="o_norm", name=f"o_norm{pair}")
                    for h2 in range(2):
                        for qch in range(TQ // 512):
                            for tok in emit_scores_exp(pair, h2, qch):
                                if isinstance(tok, tuple):
                                    flush_pending()
                                    pending[0] = (pair, h2, qch, tok[4])
                                else:
                                    ycnt[0] += 1
                                    if ycnt[0] % 2 == 0:
                                        pull_qk(pair, 1)
                flush_pending()

            # ============ Stage D: oT split, Wo (3-term fp8), residual, LN2 ====
            with contextlib.ExitStack() as dstk:
                x2p = dstk.enter_context(tc.tile_pool(name="x2p", bufs=1))
                xn2p = dstk.enter_context(tc.tile_pool(name="xn2p", bufs=2))
                x2 = x2p.tile([128, TQ // 128, C], F32R, tag="x2")
                xn2_hi = [xn2p.tile([128, NKO, 512], E4, tag="xn2hi",
                                    name=f"xn2hi{i}") for i in range(2)]
                xn2_lo = [xn2p.tile([128, NKO, 512], E5, tag="xn2lo",
                                    name=f"xn2lo{i}") for i in range(2)]

                pst2 = dstk.enter_context(tc.tile_pool(name="pst2", bufs=3,
                                                       space="PSUM"))
                psE = dstk.enter_context(tc.tile_pool(name="psE", bufs=3,
                                                      space="PSUM"))
                with contextlib.ExitStack() as dd:
                    aop = dd.enter_context(tc.tile_pool(name="aop", bufs=8))
                    wop = dd.enter_context(tc.tile_pool(name="wop", bufs=1))
                    workD = dd.enter_context(tc.tile_pool(name="workD", bufs=3))
                    wo_hi = wop.tile([128, NKO, C], E4, tag="wohi")
                    wo_lo = wop.tile([128, NKO, C], E5, tag="wolo")
                    nc.sync.dma_start(out=wo_hi[:],
                                      in_=woh_d.rearrange("(o p) f -> p o f", p=128))
                    nc.sync.dma_start(out=wo_lo[:],
                                      in_=wol_d.rearrange("(o p) f -> p o f", p=128))
                    aoT = [aop.tile([128, TQ], BF16, tag="aoT",
                                    name=f"aoT{i}") for i in range(NKO)]

                    # Wo: aoT[f, t] = sum_c oT[c, t] * wo[c, f]  (3-term fp8)
                    for f in range(NKO):
                        for ch in range(TQ // 512):
                            pw = psE.tile([128, 512], F32, tag="psE")
                            for kop in range(NKO // 2):
                                ksl = slice(2 * kop, 2 * kop + 2)
                                fsl = slice(f * 128, (f + 1) * 128)
                                nc.tensor.matmul(pw[:], wo_hi[:, ksl, fsl],
                                                 oT_hi[ch][:, ksl, :], perf_mode=DR,
                                                 start=(kop == 0), stop=False)
                                nc.tensor.matmul(pw[:], wo_lo[:, ksl, fsl],
                                                 oT_hi[ch][:, ksl, :], perf_mode=DR,
                                                 start=False, stop=False)
                                nc.tensor.matmul(pw[:], wo_hi[:, ksl, fsl],
                                                 oT_lo[ch][:, ksl, :], perf_mode=DR,
                                                 start=False, stop=(kop == NKO // 2 - 1))
                            nc.scalar.activation(
                                out=aoT[f][:, ch * 512:(ch + 1) * 512], in_=pw[:],
                                func=mybir.ActivationFunctionType.Identity,
                                bias=bo_s[:, f:f + 1], scale=1.0 / WSCALE)

                    # aoT back to token-major + residual -> x2; LN2 -> xn2 hi/lo
                    def finishD(t, mv, rstd):
                        xn2_r = workD.tile([128, C], F32R, tag="xn2_r")
                        _ln_stats_b(nc, mv, rstd, x2[:, t, :], xn2_r[:],
                                    pool=(t % 2 == 0))
                        for cg in range(2):
                            pt = pst2.tile([128, 4, 128], F32R, tag="pst2")
                            for i in range(4):
                                c = 4 * cg + i
                                nc.tensor.transpose(
                                    pt[:, i, :],
                                    xn2_r[:, c * 128:(c + 1) * 128], ident_r[:])
                            xsl = (slice(4 * cg, 4 * cg + 4),
                                   slice((t % 4) * 128, (t % 4 + 1) * 128))
                            nc.scalar.activation(
                                out=xn2_hi[t // 4][:, xsl[0], xsl[1]],
                                in_=pt[:], func=mybir.ActivationFunctionType.Copy,
                                bias=0.0, scale=1.0)
                            nc.vector.scalar_tensor_tensor(
                                out=xn2_lo[t // 4][:, xsl[0], xsl[1]],
                                in0=pt[:], scalar=1.0,
                                in1=xn2_hi[t // 4][:, xsl[0], xsl[1]],
                                op0=mybir.AluOpType.mult,
                                op1=mybir.AluOpType.subtract)

                    prevD = None
                    for t in range(TQ // 128):
                        x_t = workD.tile([128, C], F32R, tag="x_t")
                        nc.sync.dma_start(out=x_t[:],
                                          in_=x_d[t * 128:(t + 1) * 128, :].bitcast(F32R))
                        for cg in range(2):
                            pt = pst2.tile([128, 4, 128], F32, tag="pst2")
                            nc.tensor.matmul(
                                pt[:].rearrange("p a b -> p (a b)"), ident_r[:],
                                x_t[:, cg * 512:(cg + 1) * 512],
                                start=True, stop=False, skip_group_check=True)
                            for i in range(4):
                                c = 4 * cg + i
                                nc.tensor.matmul(
                                    pt[:, i, :],
                                    aoT[c][:, t * 128:(t + 1) * 128],
                                    ident_b[:], start=False, stop=(i == 3),
                                    skip_group_check=True)
                            nc.scalar.activation(
                                out=x2[:, t, cg * 512:(cg + 1) * 512],
                                in_=pt[:].rearrange("p a b -> p (a b)"),
                                func=mybir.ActivationFunctionType.Copy,
                                bias=0.0, scale=1.0)
                        mv, rstd = _ln_stats_a(nc, stats, x2[:, t, :], eps_t)
                        if prevD is not None:
                            finishD(*prevD)
                        prevD = (t, mv, rstd)
                    finishD(*prevD)

                # ============ Stage E: FFN up (W1, relu) 3-term fp8 ============
                h1p = dstk.enter_context(tc.tile_pool(name="h1p", bufs=1))
                h1_hi = h1p.tile([128, DFF // 128, TQ], E4, tag="h1hi")
                h1_lo = h1p.tile([128, DFF // 128, TQ], E5, tag="h1lo")
                w1h_r = w1h_d.rearrange("(o p) f -> p o f", p=128)
                w1l_r = w1l_d.rearrange("(o p) f -> p o f", p=128)
                with tc.tile_pool(name="w1p", bufs=2) as w1p:
                    for blk in range(DFF // 512):
                        w1th = w1p.tile([128, NKO, 512], E4, tag="w1th")
                        w1tl = w1p.tile([128, NKO, 512], E5, tag="w1tl")
                        nc.sync.dma_start(out=w1th[:],
                                          in_=w1h_r[:, :, blk * 512:(blk + 1) * 512])
                        nc.sync.dma_start(out=w1tl[:],
                                          in_=w1l_r[:, :, blk * 512:(blk + 1) * 512])
                        for ch in range(TQ // 512):
                            csl = slice(ch * 512, (ch + 1) * 512)
                            for fs in range(4):
                                f = blk * 4 + fs
                                fsl = slice(fs * 128, (fs + 1) * 128)
                                ph = psE.tile([128, 512], F32, tag="psE")
                                for kop in range(NKO // 2):
                                    ksl = slice(2 * kop, 2 * kop + 2)
                                    nc.tensor.matmul(ph[:], w1th[:, ksl, fsl],
                                                     xn2_hi[ch][:, ksl, :], perf_mode=DR,
                                                     start=(kop == 0), stop=False)
                                    nc.tensor.matmul(ph[:], w1tl[:, ksl, fsl],
                                                     xn2_hi[ch][:, ksl, :], perf_mode=DR,
                                                     start=False, stop=False)
                                    nc.tensor.matmul(ph[:], w1th[:, ksl, fsl],
                                                     xn2_lo[ch][:, ksl, :], perf_mode=DR,
                                                     start=False,
                                                     stop=(kop == NKO // 2 - 1))
                                nc.scalar.activation(
                                    out=h1_hi[:, f, csl], in_=ph[:],
                                    func=mybir.ActivationFunctionType.Relu,
                                    bias=b1_s[:, f:f + 1], scale=1.0)
                                nc.vector.scalar_tensor_tensor(
                                    out=h1_lo[:, f, csl], in0=ph[:], scalar=0.0,
                                    in1=h1_hi[:, f, csl],
                                    op0=mybir.AluOpType.max,
                                    op1=mybir.AluOpType.subtract)

                # ============ Stage F: FFN down (W2) 3-term fp8 + residual ======
                fp = dstk.enter_context(tc.tile_pool(name="fp", bufs=8))
                ffnT = [fp.tile([128, TQ], BF16, tag="ffnT",
                                name=f"ffnT{i}") for i in range(NKO)]
                w2h_r = w2h_d.rearrange("(o p) f -> p o f", p=128)
                w2l_r = w2l_d.rearrange("(o p) f -> p o f", p=128)
                with tc.tile_pool(name="w2p", bufs=2) as w2p:
                    for f in range(NKO):
                        fsl = slice(f * 128, (f + 1) * 128)
                        w2th = w2p.tile([128, DFF // 128, 128], E4, tag="w2th")
                        w2tl = w2p.tile([128, DFF // 128, 128], E5, tag="w2tl")
                        nc.sync.dma_start(out=w2th[:], in_=w2h_r[:, :, fsl])
                        nc.sync.dma_start(out=w2tl[:], in_=w2l_r[:, :, fsl])
                        for ch in range(TQ // 512):
                            csl = slice(ch * 512, (ch + 1) * 512)
                            po2 = psE.tile([128, 512], F32, tag="psE")
                            for kop in range(DFF // 256):
                                ksl = slice(2 * kop, 2 * kop + 2)
                                nc.tensor.matmul(po2[:], w2th[:, ksl, :],
                                                 h1_hi[:, ksl, csl], perf_mode=DR,
                                                 start=(kop == 0), stop=False)
                                nc.tensor.matmul(po2[:], w2tl[:, ksl, :],
                                                 h1_hi[:, ksl, csl], perf_mode=DR,
                                                 start=False, stop=False)
                                nc.tensor.matmul(po2[:], w2th[:, ksl, :],
                                                 h1_lo[:, ksl, csl], perf_mode=DR,
                                                 start=False,
                                                 stop=(kop == DFF // 256 - 1))
                            nc.scalar.activation(
                                out=ffnT[f][:, csl], in_=po2[:],
                                func=mybir.ActivationFunctionType.Identity,
                                bias=b2_s[:, f:f + 1],
                                scale=1.0 / (WSCALE * WSCALE))
                with tc.tile_pool(name="workF", bufs=2) as workF:
                    for t in range(TQ // 128):
                        out_t = workF.tile([128, C], F32, tag="out_t")
                        for cg in range(2):
                            pt = pst2.tile([128, 4, 128], F32, tag="pst2")
                            nc.tensor.matmul(
                                pt[:].rearrange("p a b -> p (a b)"), ident_r[:],
                                x2[:, t, cg * 512:(cg + 1) * 512],
                                start=True, stop=False, skip_group_check=True)
                            for i in range(4):
                                c = 4 * cg + i
                                nc.tensor.matmul(
                                    pt[:, i, :],
                                    ffnT[c][:, t * 128:(t + 1) * 128],
                                    ident_b[:], start=False, stop=(i == 3),
                                    skip_group_check=True)
                            nc.scalar.activation(
                                out=out_t[:, cg * 512:(cg + 1) * 512],
                                in_=pt[:].rearrange("p a b -> p (a b)"),
                                func=mybir.ActivationFunctionType.Copy,
                                bias=0.0, scale=1.0)
                            nc.sync.dma_start(
                                out=out_d[t * 128:(t + 1) * 128,
                                          cg * 512:(cg + 1) * 512],
                                in_=out_t[:, cg * 512:(cg + 1) * 512])

    nc.finalize()
    _legalize_sem_waits(nc)
    return nc


_NC_CACHE = None


def _get_nc():
    global _NC_CACHE
    if _NC_CACHE is None:
        _NC_CACHE = _build_nc()
    return _NC_CACHE


def _split_w(w, scale=WSCALE):
    ws = np.asarray(w, np.float32) * scale
    hi = ws.astype(ml_dtypes.float8_e4m3)
    lo = (ws - hi.astype(np.float32)).astype(ml_dtypes.float8_e5m2)
    return np.ascontiguousarray(hi), np.ascontiguousarray(lo)


def _shard_inputs(inputs):
    x = np.asarray(inputs["x"], np.float32)
    ln1_g = np.asarray(inputs["ln1_g"], np.float32).reshape(C)
    ln1_b = np.asarray(inputs["ln1_b"], np.float32).reshape(C)
    ln2_g = np.asarray(inputs["ln2_g"], np.float32).reshape(C)
    ln2_b = np.asarray(inputs["ln2_b"], np.float32).reshape(C)
    wq = np.ascontiguousarray(
        np.transpose(np.asarray(inputs["Wq"], np.float32), (1, 0, 2)).reshape(C, C))
    wk = np.ascontiguousarray(
        np.transpose(np.asarray(inputs["Wk"], np.float32), (1, 0, 2)).reshape(C, C))
    wv = np.ascontiguousarray(
        np.transpose(np.asarray(inputs["Wv"], np.float32), (1, 0, 2)).reshape(C, C))
    wo = np.asarray(inputs["Wo"], np.float32)
    w1 = np.asarray(inputs["W1"], np.float32)
    w2 = np.asarray(inputs["W2"], np.float32)

    # fold LN affine into the consuming weights/biases
    bq = np.asarray(inputs["bq"], np.float32).reshape(C) + ln1_b @ wq
    bk = np.asarray(inputs["bk"], np.float32).reshape(C) + ln1_b @ wk
    bv = np.asarray(inputs["bv"], np.float32).reshape(C) + ln1_b @ wv
    wq = np.ascontiguousarray(ln1_g[:, None] * wq)
    wk = np.ascontiguousarray(ln1_g[:, None] * wk)
    wv = np.ascontiguousarray(ln1_g[:, None] * wv)
    b1 = WSCALE * (np.asarray(inputs["b1"], np.float32).reshape(DFF) + ln2_b @ w1)
    assert np.abs(b1).max() == 0.0, "nonzero effective W1 bias unsupported by lo-split"
    w1g = ln2_g[:, None] * w1

    woh, wol = _split_w(wo)
    w1h, w1l = _split_w(w1g)
    w2h, w2l = _split_w(w2)

    shared = {
        "wq": wq, "wk": wk, "wv": wv,
        "woh": woh, "wol": wol, "w1h": w1h, "w1l": w1l, "w2h": w2h, "w2l": w2l,
        "bq": bq, "bk": bk, "bv": bv.astype(ml_dtypes.bfloat16),
        "bo": np.asarray(inputs["bo"], np.float32).reshape(C),
        "b1": b1,
        "b2": np.asarray(inputs["b2"], np.float32).reshape(C),
    }
    in_maps = []
    for c in range(N_CORES):
        b, half = c // 2, c % 2
        own = x[b, half * TQ:(half + 1) * TQ]
        other = x[b, (1 - half) * TQ:(2 - half) * TQ]
        x_perm = np.ascontiguousarray(np.concatenate([own, other], axis=0))
        in_maps.append(dict(shared, x=x_perm))
    return in_maps


def _run(inputs, **spmd_kwargs):
    nc = _get_nc()
    in_maps = _shard_inputs(inputs)
    res = run_bass_kernel_spmd(nc, in_maps, core_ids=list(range(N_CORES)), **spmd_kwargs)
    out = np.empty((B, T, C), np.float32)
    for c in range(N_CORES):
        b, half = c // 2, c % 2
        out[b, half * TQ:(half + 1) * TQ] = res.results[c]["out"]
    return out, res


def kernel(**inputs) -> np.ndarray:
    out, _ = _run(inputs)
    return out


# revision 62
# speedup vs baseline: 1.0275x; 1.0073x over previous
"""Trainium2 Bass kernel for a pre-norm transformer block (MHSA + FFN).

Sharding: 8 cores, data parallel over (batch, seq-half). Core c handles
batch c//2, sequence half c%2. Inputs are permuted so each core's own
1024 tokens come first; attention K/V run over all 2048 tokens of the
batch (softmax is permutation invariant).

Numerics: Q/K projections and scores in f32r (softmax logits are
~N(0,26^2) — fp8 there flips argmaxes). Softmax probs in bf16 with a
constant exp shift; PV runs probs-stationary so only the 65-wide
(dh+denominator) V operand streams. Wo/W1/W2 run as 3-term compensated
fp8 DoubleRow (hi=e4m3, lo=e5m2, x@W ~= xh@Wh + xh@Wl + xl@Wh), with
weights pre-scaled x32 and split host-side. LayerNorm gains/biases are
folded into the downstream weights/biases host-side, so on-chip LN is
pure z-normalization and the hi/lo activation splits are single
scalar_tensor_tensor ops.
"""
import contextlib

import numpy as np
import ml_dtypes

import concourse.bass as bass
import concourse.tile as tile
import concourse.mybir as mybir
from concourse.bass_utils import run_bass_kernel_spmd
from concourse.masks import make_identity

B, T, C = 4, 2048, 1024
H, DH = 16, 64
DFF = 4 * C
N_CORES = 8
TQ = T // 2          # tokens owned per core
TS = T               # key/value tokens per core
NKO = C // 128       # 8 contraction tiles for C
F32R = mybir.dt.float32r
F32 = mybir.dt.float32
BF16 = mybir.dt.bfloat16
E4 = mybir.dt.float8e4
E5 = mybir.dt.float8e5
EXP_BIAS = -128.0
EPS = 1e-5
WSCALE = 32.0
DR = mybir.MatmulPerfMode.DoubleRow

# ---------------------------------------------------------------------------
# Compat: this walrus build accepts at most 1 sem-wait per regular
# instruction (2 per InstEventSemaphore). bacc misses some tile-generated
# instructions, so split waits ourselves after finalize.
_ev_counter = [0]


def _legalize_sem_waits(nc):
    for func in nc.m.functions:
        for bb in func.blocks:
            new = []
            changed = False
            for inst in bb.instructions:
                si = inst.sync_info
                cap = 2 if isinstance(inst, mybir.InstEventSemaphore) else 1
                if si is not None and len(si.on_wait) > cap:
                    waits = list(si.on_wait)
                    for i in range(cap, len(waits), 2):
                        _ev_counter[0] += 1
                        e = mybir.InstEventSemaphore(
                            name=f"EVSPLIT-{_ev_counter[0]}", ins=[], outs=[])
                        e.engine = inst.engine
                        e.sync_info = mybir.SyncInfo(
                            on_wait=waits[i:i + 2], on_update=[])
                        new.append(e)
                    inst.sync_info = mybir.SyncInfo(
                        on_wait=waits[:cap], on_update=list(si.on_update))
                    changed = True
                new.append(inst)
            if changed:
                bb.instructions = new


# ---------------------------------------------------------------------------

def _ln_stats_a(nc, stats, x_ap, eps_t):
    """bn stats + sqrt(var+eps) for x_ap [128, C]; returns (mv, rstd)."""
    st = stats.tile([128, 2, 6], F32, tag="bnstats")
    mv = stats.tile([128, 2], F32, tag="bnaggr")
    xg = x_ap.rearrange("p (s d) -> p s d", s=2)
    for s in range(2):
        nc.vector.bn_stats(out=st[:, s, :], in_=xg[:, s, :])
    nc.vector.bn_aggr(out=mv[:], in_=st[:])
    rstd = stats.tile([128, 1], F32, tag="rstd")
    nc.scalar.activation(out=rstd[:], in_=mv[:, 1:2],
                         func=mybir.ActivationFunctionType.Sqrt,
                         bias=eps_t[:], scale=1.0)
    return mv, rstd


def _ln_stats_b(nc, mv, rstd, x_ap, out_ap, pool=False):
    """finish z-normalize: recip + (x - mu) * rstd."""
    nc.vector.reciprocal(out=rstd[:], in_=rstd[:])
    eng = nc.gpsimd if pool else nc.vector
    eng.tensor_scalar(out=out_ap, in0=x_ap,
                      scalar1=mv[:, 0:1], scalar2=rstd[:],
                      op0=mybir.AluOpType.subtract,
                      op1=mybir.AluOpType.mult)


def _bcast0(ap, free):
    """Broadcast a [128, n] AP along a new stride-0 free dim of size `free`."""
    return bass.AP(tensor=ap.tensor, offset=ap.offset,
                   ap=[list(d) for d in ap.ap] + [[0, free]])


def _build_nc():
    nc = bass.Bass()

    # ---- I/O ----
    x_d = nc.dram_tensor("x", [T, C], F32, kind="ExternalInput")
    wq_d = nc.dram_tensor("wq", [C, C], F32R, kind="ExternalInput")
    wk_d = nc.dram_tensor("wk", [C, C], F32R, kind="ExternalInput")
    wv_d = nc.dram_tensor("wv", [C, C], F32R, kind="ExternalInput")
    woh_d = nc.dram_tensor("woh", [C, C], E4, kind="ExternalInput")
    wol_d = nc.dram_tensor("wol", [C, C], E5, kind="ExternalInput")
    w1h_d = nc.dram_tensor("w1h", [C, DFF], E4, kind="ExternalInput")
    w1l_d = nc.dram_tensor("w1l", [C, DFF], E5, kind="ExternalInput")
    w2h_d = nc.dram_tensor("w2h", [DFF, C], E4, kind="ExternalInput")
    w2l_d = nc.dram_tensor("w2l", [DFF, C], E5, kind="ExternalInput")
    bq_d = nc.dram_tensor("bq", [C], F32, kind="ExternalInput")
    bk_d = nc.dram_tensor("bk", [C], F32, kind="ExternalInput")
    bv_d = nc.dram_tensor("bv", [C], BF16, kind="ExternalInput")
    bo_d = nc.dram_tensor("bo", [C], F32, kind="ExternalInput")
    b1_d = nc.dram_tensor("b1", [DFF], F32, kind="ExternalInput")
    b2_d = nc.dram_tensor("b2", [C], F32, kind="ExternalInput")
    out_d = nc.dram_tensor("out", [TQ, C], F32, kind="ExternalOutput")

    wq_r = wq_d.rearrange("(o p) f -> p o f", p=128)
    wk_r = wk_d.rearrange("(o p) f -> p o f", p=128)
    wv_r = wv_d.rearrange("(o p) f -> p o f", p=128)

    with tile.TileContext(nc) as tc:
        with contextlib.ExitStack() as top:
            consts = top.enter_context(tc.tile_pool(name="consts", bufs=1))
            stats = top.enter_context(tc.tile_pool(name="stats", bufs=8))
            ps = top.enter_context(tc.tile_pool(name="ps", bufs=2, space="PSUM"))

            ident_b = consts.tile([128, 128], BF16, tag="identb")
            make_identity(nc, ident_b)
            ident_r = consts.tile([128, 128], F32R, tag="identr")
            nc.vector.tensor_copy(out=ident_r[:], in_=ident_b[:])
            ebias = consts.tile([128, 1], F32, tag="ebias")
            nc.vector.memset(ebias[:], EXP_BIAS)
            eps_t = consts.tile([128, 1], F32, tag="eps")
            nc.vector.memset(eps_t[:], EPS)
            bq_s = consts.tile([128, NKO], F32, tag="bq")
            bk_s = consts.tile([128, NKO], F32, tag="bk")
            bo_s = consts.tile([128, NKO], F32, tag="bo")
            b2_s = consts.tile([128, NKO], F32, tag="b2")
            b1_s = consts.tile([128, DFF // 128], F32, tag="b1")
            for dst, src in ((bq_s, bq_d), (bk_s, bk_d), (bo_s, bo_d),
                             (b2_s, b2_d), (b1_s, b1_d)):
                nc.sync.dma_start(out=dst[:], in_=src.rearrange("(o p) -> p o", p=128))
            bv_r = consts.tile([128, C], BF16, tag="bvr")
            nc.gpsimd.dma_start(
                out=bv_r[:],
                in_=bass.AP(tensor=bv_d[:].tensor, offset=bv_d[:].offset,
                            ap=[[0, 128]] + [list(d) for d in bv_d[:].ap]))

            # ============ Stages A-C: LN1, QKV, attention ============
            with contextlib.ExitStack() as abc:
                xnp = abc.enter_context(tc.tile_pool(name="xnp", bufs=4))
                xnT_blks = [xnp.tile([128, NKO, 512], F32R, tag="xnT",
                                     name=f"xnT{i}") for i in range(4)]

                # ---- Stage A: LN1 (z-norm only) + transpose -> xnT ----
                with tc.tile_pool(name="workA", bufs=6) as workA, \
                     tc.tile_pool(name="pstA", bufs=2, space="PSUM") as pstA:
                    def finishA(t, x_t, mv, rstd):
                        xn_r = workA.tile([128, C], F32R, tag="xn_r")
                        _ln_stats_b(nc, mv, rstd, x_t[:], xn_r[:],
                                    pool=(t % 2 == 0))
                        for cg in range(2):
                            pt = pstA.tile([128, 4, 128], F32R, tag="pstA")
                            for i in range(4):
                                nc.tensor.transpose(
                                    pt[:, i, :],
                                    xn_r[:, (4 * cg + i) * 128:(4 * cg + i + 1) * 128],
                                    ident_r[:])
                            nc.scalar.activation(
                                out=xnT_blks[t // 4][:, 4 * cg:4 * cg + 4,
                                                     (t % 4) * 128:(t % 4 + 1) * 128],
                                in_=pt[:],
                                func=mybir.ActivationFunctionType.Copy,
                                bias=0.0, scale=1.0)

                    prevA = None
                    for t in range(T // 128):
                        x_t = workA.tile([128, C], F32, tag="x_t")
                        nc.sync.dma_start(out=x_t[:], in_=x_d[t * 128:(t + 1) * 128, :])
                        mv, rstd = _ln_stats_a(nc, stats, x_t[:], eps_t)
                        if prevA is not None:
                            finishA(*prevA)
                        prevA = (t, x_t, mv, rstd)
                    finishA(*prevA)

                # ---- Stages B+C interleaved ----
                wgp = abc.enter_context(tc.tile_pool(name="wgp", bufs=1))
                qkp = abc.enter_context(tc.tile_pool(name="qkp", bufs=2))
                vgp = abc.enter_context(tc.tile_pool(name="vgp", bufs=2))
                prb = abc.enter_context(tc.tile_pool(name="probs", bufs=13))
                onp = abc.enter_context(tc.tile_pool(name="onp", bufs=2))
                otp = abc.enter_context(tc.tile_pool(name="otp", bufs=2))
                pvp = abc.enter_context(tc.tile_pool(name="pvp", bufs=1, space="PSUM"))
                pso = abc.enter_context(tc.tile_pool(name="pso", bufs=1, space="PSUM"))
                ps2 = abc.enter_context(tc.tile_pool(name="ps2", bufs=2, space="PSUM"))
                asm = abc.enter_context(tc.tile_pool(name="att_sm", bufs=3))
                schp = abc.enter_context(tc.tile_pool(name="schp", bufs=2))

                oT_hi = [otp.tile([128, NKO, 512], E4, tag="oThi",
                                  name=f"oThi{i}") for i in range(2)]
                oT_lo = [otp.tile([128, NKO, 512], E5, tag="oTlo",
                                  name=f"oTlo{i}") for i in range(2)]

                qk_tiles = {}
                vg_tiles = {}

                def qkv_gen(g):
                    """Q/K for pairs 2g, 2g+1. Yields after each psum group."""
                    wqt = wgp.tile([128, NKO, 256], F32R, tag="wqt")
                    wkt = wgp.tile([128, NKO, 256], F32R, tag="wkt")
                    nc.sync.dma_start(out=wqt[:], in_=wq_r[:, :, g * 256:(g + 1) * 256])
                    nc.sync.dma_start(out=wkt[:], in_=wk_r[:, :, g * 256:(g + 1) * 256])
                    for i, f in enumerate((2 * g, 2 * g + 1)):
                        qps = [qkp.tile([128, 512], F32R, tag=f"qp{i}c{ch}",
                                        name=f"qp{f}c{ch}")
                               for ch in range(TQ // 512)]
                        kps = [qkp.tile([128, 512], F32R, tag=f"kp{i}c{ch}",
                                        name=f"kp{f}c{ch}")
                               for ch in range(TS // 512)]
                        qk_tiles[2 * g + i] = (qps, kps)
                        for ch in range(TQ // 512):
                            pq = ps.tile([128, 512], F32, tag="ps")
                            for ko in range(NKO):
                                nc.tensor.matmul(pq[:], wqt[:, ko, i * 128:(i + 1) * 128],
                                                 xnT_blks[ch][:, ko, :],
                                                 start=(ko == 0), stop=(ko == NKO - 1))
                            nc.vector.tensor_scalar(
                                out=qps[ch][:], in0=pq[:],
                                scalar1=bq_s[:, f:f + 1], scalar2=None,
                                op0=mybir.AluOpType.add)
                            yield
                        for ch in range(TS // 512):
                            pk = ps.tile([128, 512], F32, tag="ps")
                            for ko in range(NKO):
                                nc.tensor.matmul(pk[:], wkt[:, ko, i * 128:(i + 1) * 128],
                                                 xnT_blks[ch][:, ko, :],
                                                 start=(ko == 0), stop=(ko == NKO - 1))
                            nc.vector.tensor_scalar(
                                out=kps[ch][:], in0=pk[:],
                                scalar1=bk_s[:, f:f + 1], scalar2=None,
                                op0=mybir.AluOpType.add)
                            yield

                def v_gen(g):
                    """V for heads 4g..4g+3 -> vg tile [128, 16, 4, 65] bf16."""
                    wvt = wgp.tile([128, NKO, 256], F32R, tag="wvt")
                    nc.sync.dma_start(out=wvt[:], in_=wv_r[:, :, g * 256:(g + 1) * 256])
                    vg = vgp.tile([128, TS // 128, 4, 65], BF16, tag="vg")
                    vg_tiles[g] = vg
                    nc.vector.memset(vg[:, :, :, DH:DH + 1], 1.0)
                    for to in range(TS // 128):
                        pv = ps.tile([128, 512], F32, tag="ps")
                        for ko in range(NKO):
                            nc.tensor.matmul(pv[0:128, 0:256],
                                             xnT_blks[to // 4][:, ko,
                                                 (to % 4) * 128:(to % 4 + 1) * 128],
                                             wvt[:, ko, :],
                                             start=(ko == 0), stop=(ko == NKO - 1))
                        nc.vector.tensor_tensor(
                            out=vg[:, to, :, 0:DH],
                            in0=pv[:, 0:256].rearrange("p (h d) -> p h d", d=DH),
                            in1=bv_r[:, g * 256:(g + 1) * 256].rearrange(
                                "p (h d) -> p h d", d=DH),
                            op=mybir.AluOpType.add)
                        yield

                def emit_scores_exp(pair, h2, qch):
                    """Scores + exp for one (head, qch) unit. Yields per ktg."""
                    qps, kps = qk_tiles[pair]
                    base = h2 * 64
                    pbt = [prb.tile([128, 2, 512], BF16, tag="probsT",
                                    name=f"pb{kg}")
                           for kg in range(TS // 256)]
                    for ktg in range(TS // 256):
                        psc = ps2.tile([128, 2, 512], F32, tag="psc")
                        for j in range(2):
                            kt = 2 * ktg + j
                            nc.tensor.matmul(
                                psc[:, j, :],
                                kps[kt // 4][base:base + DH,
                                             (kt % 4) * 128:(kt % 4 + 1) * 128],
                                qps[qch][base:base + DH, :],
                                start=True, stop=True)
                        if pair >= 5 and ktg in (1, 4):
                            # Schraudolph exp2 on DVE: bits = y*K1+K2,
                            # bitcast to f32, clamp negatives to 0
                            for j in range(2):
                                sch = schp.tile([128, 512],
                                                mybir.dt.int32, tag="sch")
                                nc.vector.tensor_scalar(
                                    out=sch[:], in0=psc[:, j, :],
                                    scalar1=96817625.34,
                                    scalar2=-484236300.5,
                                    op0=mybir.AluOpType.mult,
                                    op1=mybir.AluOpType.add)
                                nc.vector.tensor_scalar(
                                    out=pbt[ktg][:, j, :],
                                    in0=sch[:].bitcast(F32), scalar1=0.0,
                                    scalar2=None, op0=mybir.AluOpType.max)
                        else:
                            nc.scalar.activation(
                                out=pbt[ktg][:], in_=psc[:],
                                func=mybir.ActivationFunctionType.Exp,
                                scale=8.0, bias=ebias[:])
                        yield
                    yield ("unit", pair, h2, qch, pbt)

                def emit_pv_norm(pair, h2, qch, pbt, o_norm):
                    """PV + softmax-normalize for a unit whose probs are done."""
                    vg = vg_tiles[pair // 2]
                    hl = (pair * 2 + h2) % 4
                    pvt = pvp.tile([128, 4, DH + 1], F32, tag="pvt")
                    for qt in range(4):
                        for kt in range(TS // 128):
                            nc.tensor.matmul(
                                pvt[:, qt, :],
                                pbt[kt // 2][:, kt % 2,
                                             qt * 128:(qt + 1) * 128],
                                vg[:, kt, hl, :],
                                start=(kt == 0), stop=(kt == TS // 128 - 1))
                    rec = asm.tile([128, 4], F32, tag="rec")
                    nc.vector.reciprocal(out=rec[:], in_=pvt[:, :, DH])
                    nc.vector.tensor_tensor(
                        out=o_norm[:, qch * 4:qch * 4 + 4, h2, :],
                        in0=pvt[:, :, 0:DH], in1=_bcast0(rec[:], DH),
                        op=mybir.AluOpType.mult)

                def emit_oT(pair, o_norm):
                    """Transpose pair's o chunk -> oT hi/lo (c-chunk = pair)."""
                    for ch in range(2):
                        pt = pso.tile([128, 512], F32, tag="pso")
                        for i in range(4):
                            qt = 4 * ch + i
                            nc.tensor.matmul(
                                pt[:, i * 128:(i + 1) * 128],
                                o_norm[:, qt, :, :].rearrange("p h d -> p (h d)"),
                                ident_b[:], start=True, stop=True)
                        nc.vector.tensor_copy(out=oT_hi[ch][:, pair, :], in_=pt[:])
                        nc.vector.scalar_tensor_tensor(
                            out=oT_lo[ch][:, pair, :],
                            in0=pt[:], scalar=1.0,
                            in1=oT_hi[ch][:, pair, :],
                            op0=mybir.AluOpType.mult,
                            op1=mybir.AluOpType.subtract)

                def drain(gen, n=None):
                    k = 0
                    for _ in gen:
                        k += 1
                        if n is not None and k >= n:
                            return True
                    return False

                def gen_chain(g):
                    yield from qkv_gen(g)
                    yield from v_gen(g)

                drain(gen_chain(0))
                cur = [None]
                nqk = [1]

                def pull_qk(pair, n):
                    for _ in range(n):
                        if cur[0] is None and nqk[0] < 4 and nqk[0] <= pair // 2 + 1:
                            cur[0] = gen_chain(nqk[0])
                            nqk[0] += 1
                        if cur[0] is None:
                            return
                        if not drain(cur[0], 1):
                            cur[0] = None

                ycnt = [0]
                o_norms = {}
                pending = [None]  # (pair, h2, qch, pbt)

                def flush_pending():
                    if pending[0] is not None:
                        p_, h2_, qch_, pbt_ = pending[0]
                        emit_pv_norm(p_, h2_, qch_, pbt_, o_norms[p_])
                        pending[0] = None
                        if h2_ == 1 and qch_ == TQ // 512 - 1:
                            emit_oT(p_, o_norms.pop(p_))

                for pair in range(H // 2):
                    while pair not in qk_tiles or pair // 2 not in vg_tiles:
                        pull_qk(pair, 1)
                    o_norms[pair] = onp.tile([128, TQ // 128, 2, DH], BF16,
                                             tag="o_norm", name=f"o_norm{pair}")
                    for h2 in range(2):
                        for qch in range(TQ // 512):
                            for tok in emit_scores_exp(pair, h2, qch):
                                if isinstance(tok, tuple):
                                    flush_pending()
                                    pending[0] = (pair, h2, qch, tok[4])
                                else:
                                    ycnt[0] += 1
                                    if ycnt[0] % 2 == 0:
                                        pull_qk(pair, 1)
                flush_pending()

            # ============ Stage D: oT split, Wo (3-term fp8), residual, LN2 ====
            with contextlib.ExitStack() as dstk:
                x2p = dstk.enter_context(tc.tile_pool(name="x2p", bufs=1))
                xn2p = dstk.enter_context(tc.tile_pool(name="xn2p", bufs=2))
                x2 = x2p.tile([128, TQ // 128, C], F32R, tag="x2")
                xn2_hi = [xn2p.tile([128, NKO, 512], E4, tag="xn2hi",
                                    name=f"xn2hi{i}") for i in range(2)]
                xn2_lo = [xn2p.tile([128, NKO, 512], E5, tag="xn2lo",
                                    name=f"xn2lo{i}") for i in range(2)]

                pst2 = dstk.enter_context(tc.tile_pool(name="pst2", bufs=3,
                                                       space="PSUM"))
                psE = dstk.enter_context(tc.tile_pool(name="psE", bufs=3,
                                                      space="PSUM"))
                with contextlib.ExitStack() as dd:
                    aop = dd.enter_context(tc.tile_pool(name="aop", bufs=8))
                    wop = dd.enter_context(tc.tile_pool(name="wop", bufs=1))
                    workD = dd.enter_context(tc.tile_pool(name="workD", bufs=4))
                    wo_hi = wop.tile([128, NKO, C], E4, tag="wohi")
                    wo_lo = wop.tile([128, NKO, C], E5, tag="wolo")
                    nc.sync.dma_start(out=wo_hi[:],
                                      in_=woh_d.rearrange("(o p) f -> p o f", p=128))
                    nc.sync.dma_start(out=wo_lo[:],
                                      in_=wol_d.rearrange("(o p) f -> p o f", p=128))
                    aoT = [aop.tile([128, TQ], BF16, tag="aoT",
                                    name=f"aoT{i}") for i in range(NKO)]

                    # Wo: aoT[f, t] = sum_c oT[c, t] * wo[c, f]  (3-term fp8)
                    for f in range(NKO):
                        for ch in range(TQ // 512):
                            pw = psE.tile([128, 512], F32, tag="psE")
                            for kop in range(NKO // 2):
                                ksl = slice(2 * kop, 2 * kop + 2)
                                fsl = slice(f * 128, (f + 1) * 128)
                                nc.tensor.matmul(pw[:], wo_hi[:, ksl, fsl],
                                                 oT_hi[ch][:, ksl, :], perf_mode=DR,
                                                 start=(kop == 0), stop=False)
                                nc.tensor.matmul(pw[:], wo_lo[:, ksl, fsl],
                                                 oT_hi[ch][:, ksl, :], perf_mode=DR,
                                                 start=False, stop=False)
                                nc.tensor.matmul(pw[:], wo_hi[:, ksl, fsl],
                                                 oT_lo[ch][:, ksl, :], perf_mode=DR,
                                                 start=False, stop=(kop == NKO // 2 - 1))
                            nc.scalar.activation(
                                out=aoT[f][:, ch * 512:(ch + 1) * 512], in_=pw[:],
                                func=mybir.ActivationFunctionType.Identity,
                                bias=bo_s[:, f:f + 1], scale=1.0 / WSCALE)

                    # aoT back to token-major + residual -> x2; LN2 -> xn2 hi/lo
                    def finishD(t, mv, rstd):
                        xn2_r = workD.tile([128, C], F32R, tag="xn2_r")
                        _ln_stats_b(nc, mv, rstd, x2[:, t, :], xn2_r[:],
                                    pool=(t % 2 == 0))
                        for cg in range(2):
                            pt = pst2.tile([128, 4, 128], F32R, tag="pst2")
                            for i in range(4):
                                c = 4 * cg + i
                                nc.tensor.transpose(
                                    pt[:, i, :],
                                    xn2_r[:, c * 128:(c + 1) * 128], ident_r[:])
                            xsl = (slice(4 * cg, 4 * cg + 4),
                                   slice((t % 4) * 128, (t % 4 + 1) * 128))
                            nc.scalar.activation(
                                out=xn2_hi[t // 4][:, xsl[0], xsl[1]],
                                in_=pt[:], func=mybir.ActivationFunctionType.Copy,
                                bias=0.0, scale=1.0)
                            nc.vector.scalar_tensor_tensor(
                                out=xn2_lo[t // 4][:, xsl[0], xsl[1]],
                                in0=pt[:], scalar=1.0,
                                in1=xn2_hi[t // 4][:, xsl[0], xsl[1]],
                                op0=mybir.AluOpType.mult,
                                op1=mybir.AluOpType.subtract)

                    prevD = None
                    for t in range(TQ // 128):
                        x_t = workD.tile([128, C], F32R, tag="x_t")
                        nc.sync.dma_start(out=x_t[:],
                                          in_=x_d[t * 128:(t + 1) * 128, :].bitcast(F32R))
                        for cg in range(2):
                            pt = pst2.tile([128, 4, 128], F32, tag="pst2")
                            nc.tensor.matmul(
                                pt[:].rearrange("p a b -> p (a b)"), ident_r[:],
                                x_t[:, cg * 512:(cg + 1) * 512],
                                start=True, stop=False, skip_group_check=True)
                            for i in range(4):
                                c = 4 * cg + i
                                nc.tensor.matmul(
                                    pt[:, i, :],
                                    aoT[c][:, t * 128:(t + 1) * 128],
                                    ident_b[:], start=False, stop=(i == 3),
                                    skip_group_check=True)
                            nc.scalar.activation(
                                out=x2[:, t, cg * 512:(cg + 1) * 512],
                                in_=pt[:].rearrange("p a b -> p (a b)"),
                                func=mybir.ActivationFunctionType.Copy,
                                bias=0.0, scale=1.0)
                        mv, rstd = _ln_stats_a(nc, stats, x2[:, t, :], eps_t)
                        if prevD is not None:
                            finishD(*prevD)
                        prevD = (t, mv, rstd)
                    finishD(*prevD)

                # ============ Stage E: FFN up (W1, relu) 3-term fp8 ============
                h1p = dstk.enter_context(tc.tile_pool(name="h1p", bufs=1))
                h1_hi = h1p.tile([128, DFF // 128, TQ], E4, tag="h1hi")
                h1_lo = h1p.tile([128, DFF // 128, TQ], E5, tag="h1lo")
                w1h_r = w1h_d.rearrange("(o p) f -> p o f", p=128)
                w1l_r = w1l_d.rearrange("(o p) f -> p o f", p=128)
                with tc.tile_pool(name="w1p", bufs=2) as w1p:
                    for blk in range(DFF // 512):
                        w1th = w1p.tile([128, NKO, 512], E4, tag="w1th")
                        w1tl = w1p.tile([128, NKO, 512], E5, tag="w1tl")
                        nc.sync.dma_start(out=w1th[:],
                                          in_=w1h_r[:, :, blk * 512:(blk + 1) * 512])
                        nc.sync.dma_start(out=w1tl[:],
                                          in_=w1l_r[:, :, blk * 512:(blk + 1) * 512])
                        for ch in range(TQ // 512):
                            csl = slice(ch * 512, (ch + 1) * 512)
                            for fs in range(4):
                                f = blk * 4 + fs
                                fsl = slice(fs * 128, (fs + 1) * 128)
                                ph = psE.tile([128, 512], F32, tag="psE")
                                for kop in range(NKO // 2):
                                    ksl = slice(2 * kop, 2 * kop + 2)
                                    nc.tensor.matmul(ph[:], w1th[:, ksl, fsl],
                                                     xn2_hi[ch][:, ksl, :], perf_mode=DR,
                                                     start=(kop == 0), stop=False)
                                    nc.tensor.matmul(ph[:], w1tl[:, ksl, fsl],
                                                     xn2_hi[ch][:, ksl, :], perf_mode=DR,
                                                     start=False, stop=False)
                                    nc.tensor.matmul(ph[:], w1th[:, ksl, fsl],
                                                     xn2_lo[ch][:, ksl, :], perf_mode=DR,
                                                     start=False,
                                                     stop=(kop == NKO // 2 - 1))
                                nc.scalar.activation(
                                    out=h1_hi[:, f, csl], in_=ph[:],
                                    func=mybir.ActivationFunctionType.Relu,
                                    bias=b1_s[:, f:f + 1], scale=1.0)
                                nc.vector.scalar_tensor_tensor(
                                    out=h1_lo[:, f, csl], in0=ph[:], scalar=0.0,
                                    in1=h1_hi[:, f, csl],
                                    op0=mybir.AluOpType.max,
                                    op1=mybir.AluOpType.subtract)

                # ============ Stage F: FFN down (W2) 3-term fp8 + residual ======
                fp = dstk.enter_context(tc.tile_pool(name="fp", bufs=8))
                ffnT = [fp.tile([128, TQ], BF16, tag="ffnT",
                                name=f"ffnT{i}") for i in range(NKO)]
                w2h_r = w2h_d.rearrange("(o p) f -> p o f", p=128)
                w2l_r = w2l_d.rearrange("(o p) f -> p o f", p=128)
                with tc.tile_pool(name="w2p", bufs=2) as w2p:
                    for f in range(NKO):
                        fsl = slice(f * 128, (f + 1) * 128)
                        w2th = w2p.tile([128, DFF // 128, 128], E4, tag="w2th")
                        w2tl = w2p.tile([128, DFF // 128, 128], E5, tag="w2tl")
                        nc.sync.dma_start(out=w2th[:], in_=w2h_r[:, :, fsl])
                        nc.sync.dma_start(out=w2tl[:], in_=w2l_r[:, :, fsl])
                        for ch in range(TQ // 512):
                            csl = slice(ch * 512, (ch + 1) * 512)
                            po2 = psE.tile([128, 512], F32, tag="psE")
                            for kop in range(DFF // 256):
                                ksl = slice(2 * kop, 2 * kop + 2)
                                nc.tensor.matmul(po2[:], w2th[:, ksl, :],
                                                 h1_hi[:, ksl, csl], perf_mode=DR,
                                                 start=(kop == 0), stop=False)
                                nc.tensor.matmul(po2[:], w2tl[:, ksl, :],
                                                 h1_hi[:, ksl, csl], perf_mode=DR,
                                                 start=False, stop=False)
                                nc.tensor.matmul(po2[:], w2th[:, ksl, :],
                                                 h1_lo[:, ksl, csl], perf_mode=DR,
                                                 start=False,
                                                 stop=(kop == DFF // 256 - 1))
                            nc.scalar.activation(
                                out=ffnT[f][:, csl], in_=po2[:],
                                func=mybir.ActivationFunctionType.Identity,
                                bias=b2_s[:, f:f + 1],
                                scale=1.0 / (WSCALE * WSCALE))
                with tc.tile_pool(name="workF", bufs=3) as workF:
                    for t in range(TQ // 128):
                        out_t = workF.tile([128, C], F32, tag="out_t")
                        for cg in range(2):
                            pt = pst2.tile([128, 4, 128], F32, tag="pst2")
                            nc.tensor.matmul(
                                pt[:].rearrange("p a b -> p (a b)"), ident_r[:],
                                x2[:, t, cg * 512:(cg + 1) * 512],
                                start=True, stop=False, skip_group_check=True)
                            for i in range(4):
                                c = 4 * cg + i
                                nc.tensor.matmul(
                                    pt[:, i, :],
                                    ffnT[c][:, t * 128:(t + 1) * 128],
                                    ident_b[:], start=False, stop=(i == 3),
                                    skip_group_check=True)
                            nc.scalar.activation(
                                out=out_t[:, cg * 512:(cg + 1) * 512],
                                in_=pt[:].rearrange("p a b -> p (a b)"),
                                func=mybir.ActivationFunctionType.Copy,
                                bias=0.0, scale=1.0)
                            nc.sync.dma_start(
                                out=out_d[t * 128:(t + 1) * 128,
                                          cg * 512:(cg + 1) * 512],
                                in_=out_t[:, cg * 512:(cg + 1) * 512])

    nc.finalize()
    _legalize_sem_waits(nc)
    return nc


_NC_CACHE = None


def _get_nc():
    global _NC_CACHE
    if _NC_CACHE is None:
        _NC_CACHE = _build_nc()
    return _NC_CACHE


def _split_w(w, scale=WSCALE):
    ws = np.asarray(w, np.float32) * scale
    hi = ws.astype(ml_dtypes.float8_e4m3)
    lo = (ws - hi.astype(np.float32)).astype(ml_dtypes.float8_e5m2)
    return np.ascontiguousarray(hi), np.ascontiguousarray(lo)


def _shard_inputs(inputs):
    x = np.asarray(inputs["x"], np.float32)
    ln1_g = np.asarray(inputs["ln1_g"], np.float32).reshape(C)
    ln1_b = np.asarray(inputs["ln1_b"], np.float32).reshape(C)
    ln2_g = np.asarray(inputs["ln2_g"], np.float32).reshape(C)
    ln2_b = np.asarray(inputs["ln2_b"], np.float32).reshape(C)
    wq = np.ascontiguousarray(
        np.transpose(np.asarray(inputs["Wq"], np.float32), (1, 0, 2)).reshape(C, C))
    wk = np.ascontiguousarray(
        np.transpose(np.asarray(inputs["Wk"], np.float32), (1, 0, 2)).reshape(C, C))
    wv = np.ascontiguousarray(
        np.transpose(np.asarray(inputs["Wv"], np.float32), (1, 0, 2)).reshape(C, C))
    wo = np.asarray(inputs["Wo"], np.float32)
    w1 = np.asarray(inputs["W1"], np.float32)
    w2 = np.asarray(inputs["W2"], np.float32)

    # fold LN affine into the consuming weights/biases
    bq = np.asarray(inputs["bq"], np.float32).reshape(C) + ln1_b @ wq
    bk = np.asarray(inputs["bk"], np.float32).reshape(C) + ln1_b @ wk
    bv = np.asarray(inputs["bv"], np.float32).reshape(C) + ln1_b @ wv
    wq = np.ascontiguousarray(ln1_g[:, None] * wq)
    wk = np.ascontiguousarray(ln1_g[:, None] * wk)
    wv = np.ascontiguousarray(ln1_g[:, None] * wv)
    b1 = WSCALE * (np.asarray(inputs["b1"], np.float32).reshape(DFF) + ln2_b @ w1)
    assert np.abs(b1).max() == 0.0, "nonzero effective W1 bias unsupported by lo-split"
    w1g = ln2_g[:, None] * w1

    woh, wol = _split_w(wo)
    w1h, w1l = _split_w(w1g)
    w2h, w2l = _split_w(w2)

    shared = {
        "wq": wq, "wk": wk, "wv": wv,
        "woh": woh, "wol": wol, "w1h": w1h, "w1l": w1l, "w2h": w2h, "w2l": w2l,
        "bq": bq, "bk": bk, "bv": bv.astype(ml_dtypes.bfloat16),
        "bo": np.asarray(inputs["bo"], np.float32).reshape(C),
        "b1": b1,
        "b2": np.asarray(inputs["b2"], np.float32).reshape(C),
    }
    in_maps = []
    for c in range(N_CORES):
        b, half = c // 2, c % 2
        own = x[b, half * TQ:(half + 1) * TQ]
        other = x[b, (1 - half) * TQ:(2 - half) * TQ]
        x_perm = np.ascontiguousarray(np.concatenate([own, other], axis=0))
        in_maps.append(dict(shared, x=x_perm))
    return in_maps


def _run(inputs, **spmd_kwargs):
    nc = _get_nc()
    in_maps = _shard_inputs(inputs)
    res = run_bass_kernel_spmd(nc, in_maps, core_ids=list(range(N_CORES)), **spmd_kwargs)
    out = np.empty((B, T, C), np.float32)
    for c in range(N_CORES):
        b, half = c // 2, c % 2
        out[b, half * TQ:(half + 1) * TQ] = res.results[c]["out"]
    return out, res


def kernel(**inputs) -> np.ndarray:
    out, _ = _run(inputs)
    return out
